# revision 1
# baseline (speedup 1.0000x reference)
"""Linear-chain CRF loss (mean over batch of logZ - gold_score) on 8 TRN2 cores.

Math: the forward (alpha) recursion is run in the exp domain so each step is a
single 128x128 @ 128xW matmul on the PE plus one elementwise multiply:
    a_{t}[j,b] = ee_t[j,b] * sum_i E[i,j] * a_{t-1}[i,b]
with E = exp(transitions) kept stationary (bf16 lhsT) and
ee_t = exp(emissions[b,t,:] - MU) streamed from HBM in a host-pretransposed
(C, T, B_local) layout.  MU keeps per-step growth ~1; an exact
sum-renormalization every RENORM steps (ones-matmul -> reciprocal ->
K=1-broadcast-matmul) removes drift, accumulating log(s) into a per-b offset.
Final: logz = log(a_T . exp(end)) + sum log s + T*MU.

Sharding: data-parallel over batch, 16 sequences per core, no collectives;
host computes the (tiny) gold path score and the final mean.
"""

import numpy as np
from contextlib import ExitStack

import concourse.bass as bass
import concourse.bacc as bacc
import concourse.mybir as mybir
from concourse.tile import TileContext
from concourse import bass_utils

B, T, C = 128, 1024, 128
NCORES = 8
BLOC = B // NCORES            # 16 sequences per core
NCHAINS = 2                   # independent recursion chains per core (pipelining)
CW = BLOC // NCHAINS          # chain width (free dim of the per-step matmul)
TCH = 64                      # time steps per streamed emissions chunk
RENORM = 128                  # steps between exact renormalizations
MU = 5.9                      # per-step log-growth pre-subtraction

F32 = mybir.dt.float32
BF16 = mybir.dt.bfloat16
AF = mybir.ActivationFunctionType

_cache = {}


def _build(renorm=RENORM, psum_bufs=3, a_bufs=128):
    """Bidirectional (meet-in-the-middle) CRF forward pass: the alpha
    recursion runs t=1..T/2 while the beta recursion runs t=T-1..T/2
    concurrently — both boundary conditions are known, halving the serial
    dependence chain to T/2 links.  logZ = log sum_j alpha[j]*beta[j]."""
    key = (renorm, psum_bufs, a_bufs)
    if key in _cache:
        return _cache[key]
    cw = BLOC
    nc = bacc.Bacc("TRN2", target_bir_lowering=False, debug=False)
    em = nc.dram_tensor("em", (C, T, BLOC), F32, kind="ExternalInput")
    trans = nc.dram_tensor("trans", (C, C), F32, kind="ExternalInput")
    transT = nc.dram_tensor("transT", (C, C), F32, kind="ExternalInput")
    startv = nc.dram_tensor("startv", (C, 1), F32, kind="ExternalInput")
    endv = nc.dram_tensor("endv", (C, 1), F32, kind="ExternalInput")
    out = nc.dram_tensor("logz_out", (1, BLOC), F32, kind="ExternalOutput")

    half = T // 2
    nchunks = T // TCH
    with TileContext(nc) as tc, ExitStack() as ctx:
        consts = ctx.enter_context(tc.tile_pool(name="consts", bufs=1))
        emraw = ctx.enter_context(tc.tile_pool(name="emraw", bufs=nchunks))
        eepool = ctx.enter_context(tc.tile_pool(name="ee", bufs=nchunks))
        apool = ctx.enter_context(tc.tile_pool(name="a", bufs=a_bufs))
        small = ctx.enter_context(tc.tile_pool(name="small", bufs=40))
        ppool = ctx.enter_context(tc.tile_pool(name="psum", bufs=psum_bufs, space="PSUM"))
        rpool = ctx.enter_context(tc.tile_pool(name="rpsum", bufs=1, space="PSUM"))

        trans_sb = consts.tile([C, C], F32, tag="tr")
        nc.sync.dma_start(out=trans_sb, in_=trans[:, :])
        Ef_f = consts.tile([C, C], F32, tag="eff")
        nc.scalar.activation(Ef_f, trans_sb, AF.Exp)
        # Fold the per-step growth normalizer exp(-MU) into the stationary
        # transition matrices (avoids a bias operand on the streamed exps).
        Ef = consts.tile([C, C], BF16, tag="ef")
        nc.vector.tensor_scalar_mul(Ef, Ef_f, float(np.exp(-MU)))

        transT_sb = consts.tile([C, C], F32, tag="trT")
        nc.sync.dma_start(out=transT_sb, in_=transT[:, :])
        Eb_f = consts.tile([C, C], F32, tag="ebf")
        nc.scalar.activation(Eb_f, transT_sb, AF.Exp)
        Eb = consts.tile([C, C], BF16, tag="eb")
        nc.vector.tensor_scalar_mul(Eb, Eb_f, float(np.exp(-MU)))

        sv = consts.tile([C, 1], F32, tag="sv")
        nc.sync.dma_start(out=sv, in_=startv[:, :])
        Estart = consts.tile([C, 1], F32, tag="es")
        nc.scalar.activation(Estart, sv, AF.Exp)

        ev = consts.tile([C, 1], F32, tag="ev")
        nc.sync.dma_start(out=ev, in_=endv[:, :])
        Eend = consts.tile([C, 1], F32, tag="ee_c")
        nc.scalar.activation(Eend, ev, AF.Exp)

        ones_col = consts.tile([C, 1], BF16, tag="oc")
        nc.vector.memset(ones_col, 1.0)
        ones_row = consts.tile([1, C], F32, tag="or")
        nc.vector.memset(ones_row, 1.0)

        off_f = consts.tile([1, cw], F32, tag="off_f")
        nc.vector.memset(off_f, 0.0)
        off_b = consts.tile([1, cw], F32, tag="off_b")
        nc.vector.memset(off_b, 0.0)

        # Stream all emission chunks; order interleaves the two ends so the
        # earliest-needed chunks of each direction are first in queue order.
        ee = [None] * nchunks
        order = []
        for i in range(nchunks // 2):
            order += [i, nchunks - 1 - i]
        for ch in order:
            emt = emraw.tile([C, TCH, BLOC], F32)
            nc.gpsimd.dma_start(out=emt[:], in_=em[:, ch * TCH:(ch + 1) * TCH, :])
            e = eepool.tile([C, TCH, BLOC], BF16)
            nc.scalar.activation(e[:], emt[:], AF.Exp)
            ee[ch] = e

        def ee_at(t):
            return ee[t // TCH][:, t % TCH, :]

        def renorm_chain(state, off_acc):
            ssum = rpool.tile([1, cw], F32, tag="rs")
            nc.tensor.matmul(ssum[:], ones_col[:], state[:], start=True, stop=True)
            rcp = small.tile([1, cw], F32, tag="rcp")
            nc.vector.reciprocal(rcp, ssum)
            lg = small.tile([1, cw], F32, tag="lg")
            nc.scalar.activation(lg, ssum, AF.Ln)
            nc.vector.tensor_add(off_acc, off_acc, lg)
            bc = rpool.tile([C, cw], F32, tag="rb")
            nc.tensor.matmul(bc[:], ones_row[:], rcp[:], start=True, stop=True)
            nw = apool.tile([C, cw], BF16, tag="ren")
            nc.vector.tensor_mul(nw, state, bc)
            return nw

        # Forward init (t=0): a = ee_0 * exp(start), per-partition scalar.
        a = apool.tile([C, cw], BF16, tag="af")
        nc.vector.tensor_scalar_mul(a, ee_at(0), Estart[:, 0:1])
        # Backward init (t=T-1): w = ee_{T-1} * exp(end).
        w = apool.tile([C, cw], BF16, tag="ab")
        nc.vector.tensor_scalar_mul(w, ee_at(T - 1), Eend[:, 0:1])

        beta_ps = None
        for kk in range(half):
            # forward step t = kk+1: a <- ee_t * (Ef^T a)
            tf = kk + 1
            p = ppool.tile([C, cw], F32, tag="pf")
            nc.tensor.matmul(p[:], Ef[:], a[:], start=True, stop=True)
            an = apool.tile([C, cw], BF16, tag="af")
            nc.vector.tensor_mul(an, p, ee_at(tf))
            a = an
            # backward step kk: matmul produces beta at t = T-2-kk; the
            # following multiply applies emission T-2-kk while that emission
            # still belongs to the backward half (t >= T/2+1).
            tb = T - 2 - kk
            if tb >= half + 1:
                p2 = ppool.tile([C, cw], F32, tag="pb")
                nc.tensor.matmul(p2[:], Eb[:], w[:], start=True, stop=True)
                wn = apool.tile([C, cw], BF16, tag="ab")
                nc.vector.tensor_mul(wn, p2, ee_at(tb))
                w = wn
            elif tb == half:
                # final backward matmul yields beta_{T/2}; emission at T/2
                # belongs to the forward pass
                beta_ps = ppool.tile([C, cw], F32, tag="pb")
                nc.tensor.matmul(beta_ps[:], Eb[:], w[:], start=True, stop=True)
            if (kk + 1) % renorm == 0 and kk < half - 1:
                a = renorm_chain(a, off_f)
                w = renorm_chain(w, off_b)

        # Meet: logZ = log sum_j a[j]*beta[j] + offsets (+ MU*(T-1) on host).
        m = apool.tile([C, cw], BF16, tag="meet")
        nc.vector.tensor_mul(m, beta_ps, a)
        z = rpool.tile([1, cw], F32, tag="rs")
        nc.tensor.matmul(z[:], ones_col[:], m[:], start=True, stop=True)
        lg = small.tile([1, cw], F32, tag="lg")
        nc.scalar.activation(lg, z, AF.Ln)
        res = consts.tile([1, BLOC], F32, tag="res")
        nc.vector.tensor_add(res, lg, off_f)
        nc.vector.tensor_add(res, res, off_b)
        nc.sync.dma_start(out=out[:, :], in_=res[:])

    nc.compile()
    _cache[key] = nc
    return nc


def _gold_np(emissions, tags, mask, transitions, start_transitions, end_transitions):
    em = emissions.astype(np.float64)
    mf = mask.astype(np.float64)
    idx = np.arange(B)
    emit = np.take_along_axis(em, tags[:, :, None], axis=2)[:, :, 0]
    tr = transitions.astype(np.float64)[tags[:, :-1], tags[:, 1:]]
    score = start_transitions.astype(np.float64)[tags[:, 0]] + emit[:, 0]
    score = score + np.sum((emit[:, 1:] + tr) * mf[:, 1:], axis=1)
    last_idx = mask.astype(np.int64).sum(axis=1) - 1
    last_tags = tags[idx, last_idx]
    return score + end_transitions.astype(np.float64)[last_tags]


def _logz_host(emissions, mask, transitions, start_transitions, end_transitions):
    # Slow exact fallback (only for non-all-ones masks, which the spec never
    # produces).
    em = emissions.astype(np.float64)
    tr = transitions.astype(np.float64)
    alpha = start_transitions.astype(np.float64) + em[:, 0]
    for t in range(1, T):
        sc = alpha[:, :, None] + tr[None] + em[:, t, None, :]
        m = sc.max(axis=1)
        nxt = m + np.log(np.exp(sc - m[:, None, :]).sum(axis=1))
        alpha = np.where(mask[:, t, None], nxt, alpha)
    fin = alpha + end_transitions.astype(np.float64)[None]
    m = fin.max(axis=1)
    return m + np.log(np.exp(fin - m[:, None]).sum(axis=1))


def run_device(in_maps, trace=False, **kw):
    nc = _build()
    return bass_utils.run_bass_kernel_spmd(
        nc, in_maps, core_ids=list(range(NCORES)), trace=trace, **kw)


def make_in_maps(emissions, transitions, start_transitions, end_transitions):
    tr = np.ascontiguousarray(transitions, dtype=np.float32)
    trT = np.ascontiguousarray(transitions.T, dtype=np.float32)
    sv = np.ascontiguousarray(start_transitions, dtype=np.float32).reshape(C, 1)
    ev = np.ascontiguousarray(end_transitions, dtype=np.float32).reshape(C, 1)
    in_maps = []
    for k in range(NCORES):
        sl = slice(k * BLOC, (k + 1) * BLOC)
        em_k = np.ascontiguousarray(
            emissions[sl].transpose(2, 1, 0).astype(np.float32))
        in_maps.append({"em": em_k, "trans": tr, "transT": trT,
                        "startv": sv, "endv": ev})
    return in_maps


def kernel(**inputs):
    emissions = np.asarray(inputs["emissions"], dtype=np.float32)
    tags = np.asarray(inputs["tags"]).astype(np.int64)
    mask = np.asarray(inputs["mask"]).astype(bool)
    transitions = np.asarray(inputs["transitions"], dtype=np.float32)
    start_transitions = np.asarray(inputs["start_transitions"], dtype=np.float32)
    end_transitions = np.asarray(inputs["end_transitions"], dtype=np.float32)

    gold = _gold_np(emissions, tags, mask, transitions,
                    start_transitions, end_transitions)

    if mask.all():
        in_maps = make_in_maps(emissions, transitions,
                               start_transitions, end_transitions)
        res = run_device(in_maps)
        logz = np.concatenate([r["logz_out"][0] for r in res.results])
        # Eexp carries exp(-MU); it is applied on steps 1..T-1 only.
        logz = logz.astype(np.float64) + MU * (T - 1)
    else:
        logz = _logz_host(emissions, mask, transitions,
                          start_transitions, end_transitions)

    loss = np.mean(logz - gold)
    return np.asarray(loss, dtype=np.float32)



# revision 5
# speedup vs baseline: 1.1429x; 1.1429x over previous
"""Linear-chain CRF loss (mean over batch of logZ - gold_score) on 8 TRN2 cores.

Math: the forward (alpha) recursion is run in the exp domain so each step is a
single 128x128 @ 128xW matmul on the PE plus one elementwise multiply:
    a_{t}[j,b] = ee_t[j,b] * sum_i E[i,j] * a_{t-1}[i,b]
with E = exp(transitions) kept stationary (bf16 lhsT) and
ee_t = exp(emissions[b,t,:] - MU) streamed from HBM in a host-pretransposed
(C, T, B_local) layout.  MU keeps per-step growth ~1; an exact
sum-renormalization every RENORM steps (ones-matmul -> reciprocal ->
K=1-broadcast-matmul) removes drift, accumulating log(s) into a per-b offset.
Final: logz = log(a_T . exp(end)) + sum log s + T*MU.

Sharding: data-parallel over batch, 16 sequences per core, no collectives;
host computes the (tiny) gold path score and the final mean.
"""

import numpy as np
from contextlib import ExitStack

import concourse.bass as bass
import concourse.bacc as bacc
import concourse.mybir as mybir
from concourse.tile import TileContext
from concourse import bass_utils

B, T, C = 128, 1024, 128
NCORES = 8
BLOC = B // NCORES            # 16 sequences per core
NCHAINS = 2                   # independent recursion chains per core (pipelining)
CW = BLOC // NCHAINS          # chain width (free dim of the per-step matmul)
TCH = 64                      # time steps per streamed emissions chunk
RENORM = 4096                 # steps between exact renormalizations (off: >T)
MU = 5.9                      # per-step log-growth pre-subtraction

F32 = mybir.dt.float32
BF16 = mybir.dt.bfloat16
AF = mybir.ActivationFunctionType

_cache = {}


def _build(renorm=RENORM, psum_bufs=3, a_bufs=128):
    """Bidirectional (meet-in-the-middle) CRF forward pass: the alpha
    recursion runs t=1..T/2 while the beta recursion runs t=T-1..T/2
    concurrently — both boundary conditions are known, halving the serial
    dependence chain to T/2 links.  logZ = log sum_j alpha[j]*beta[j]."""
    key = (renorm, psum_bufs, a_bufs)
    if key in _cache:
        return _cache[key]
    cw = BLOC
    nc = bacc.Bacc("TRN2", target_bir_lowering=False, debug=False)
    em = nc.dram_tensor("em", (C, T, BLOC), F32, kind="ExternalInput")
    trans = nc.dram_tensor("trans", (C, C), F32, kind="ExternalInput")
    transT = nc.dram_tensor("transT", (C, C), F32, kind="ExternalInput")
    startv = nc.dram_tensor("startv", (C, 1), F32, kind="ExternalInput")
    endv = nc.dram_tensor("endv", (C, 1), F32, kind="ExternalInput")
    out = nc.dram_tensor("logz_out", (1, BLOC), F32, kind="ExternalOutput")

    half = T // 2
    nchunks = T // TCH
    with TileContext(nc) as tc, ExitStack() as ctx:
        consts = ctx.enter_context(tc.tile_pool(name="consts", bufs=1))
        emraw = ctx.enter_context(tc.tile_pool(name="emraw", bufs=nchunks))
        eepool = ctx.enter_context(tc.tile_pool(name="ee", bufs=nchunks))
        apool = ctx.enter_context(tc.tile_pool(name="a", bufs=a_bufs))
        small = ctx.enter_context(tc.tile_pool(name="small", bufs=40))
        ppool = ctx.enter_context(tc.tile_pool(name="psum", bufs=psum_bufs, space="PSUM"))
        rpool = ctx.enter_context(tc.tile_pool(name="rpsum", bufs=1, space="PSUM"))

        trans_sb = consts.tile([C, C], F32, tag="tr")
        nc.sync.dma_start(out=trans_sb, in_=trans[:, :])
        Ef_f = consts.tile([C, C], F32, tag="eff")
        nc.scalar.activation(Ef_f, trans_sb, AF.Exp)
        # Fold the per-step growth normalizer exp(-MU) into the stationary
        # transition matrices (avoids a bias operand on the streamed exps).
        Ef = consts.tile([C, C], BF16, tag="ef")
        nc.vector.tensor_scalar_mul(Ef, Ef_f, float(np.exp(-MU)))

        transT_sb = consts.tile([C, C], F32, tag="trT")
        nc.sync.dma_start(out=transT_sb, in_=transT[:, :])
        Eb_f = consts.tile([C, C], F32, tag="ebf")
        nc.scalar.activation(Eb_f, transT_sb, AF.Exp)
        Eb = consts.tile([C, C], BF16, tag="eb")
        nc.vector.tensor_scalar_mul(Eb, Eb_f, float(np.exp(-MU)))

        sv = consts.tile([C, 1], F32, tag="sv")
        nc.sync.dma_start(out=sv, in_=startv[:, :])
        Estart = consts.tile([C, 1], F32, tag="es")
        nc.scalar.activation(Estart, sv, AF.Exp)

        ev = consts.tile([C, 1], F32, tag="ev")
        nc.sync.dma_start(out=ev, in_=endv[:, :])
        Eend = consts.tile([C, 1], F32, tag="ee_c")
        nc.scalar.activation(Eend, ev, AF.Exp)

        ones_col = consts.tile([C, 1], BF16, tag="oc")
        nc.vector.memset(ones_col, 1.0)
        ones_row = consts.tile([1, C], F32, tag="or")
        nc.vector.memset(ones_row, 1.0)

        off_f = consts.tile([1, cw], F32, tag="off_f")
        nc.vector.memset(off_f, 0.0)
        off_b = consts.tile([1, cw], F32, tag="off_b")
        nc.vector.memset(off_b, 0.0)

        # Stream all emission chunks; order interleaves the two ends so the
        # earliest-needed chunks of each direction are first in queue order.
        ee = [None] * nchunks
        order = []
        for i in range(nchunks // 2):
            order += [i, nchunks - 1 - i]
        for ch in order:
            emt = emraw.tile([C, TCH, BLOC], F32)
            # HWDGE via SP keeps SWDGE descriptor-gen off the Pool engine,
            # which now owns the critical-path per-step multiplies.
            nc.sync.dma_start(out=emt[:], in_=em[:, ch * TCH:(ch + 1) * TCH, :])
            e = eepool.tile([C, TCH, BLOC], BF16)
            nc.scalar.activation(e[:], emt[:], AF.Exp)
            ee[ch] = e

        def ee_at(t):
            return ee[t // TCH][:, t % TCH, :]

        def renorm_chain(state, off_acc):
            ssum = rpool.tile([1, cw], F32, tag="rs")
            nc.tensor.matmul(ssum[:], ones_col[:], state[:], start=True, stop=True)
            rcp = small.tile([1, cw], F32, tag="rcp")
            nc.vector.reciprocal(rcp, ssum)
            lg = small.tile([1, cw], F32, tag="lg")
            nc.scalar.activation(lg, ssum, AF.Ln)
            nc.vector.tensor_add(off_acc, off_acc, lg)
            bc = rpool.tile([C, cw], F32, tag="rb")
            nc.tensor.matmul(bc[:], ones_row[:], rcp[:], start=True, stop=True)
            nw = apool.tile([C, cw], BF16, tag="ren")
            nc.vector.tensor_mul(nw, state, bc)
            return nw

        # Forward init (t=0): a = ee_0 * exp(start), per-partition scalar.
        a = apool.tile([C, cw], BF16, tag="af")
        nc.vector.tensor_scalar_mul(a, ee_at(0), Estart[:, 0:1])
        # Backward init (t=T-1): w = ee_{T-1} * exp(end).
        w = apool.tile([C, cw], BF16, tag="ab")
        nc.vector.tensor_scalar_mul(w, ee_at(T - 1), Eend[:, 0:1])

        beta_ps = None
        for kk in range(half):
            # forward step t = kk+1: a <- ee_t * (Ef^T a)
            tf = kk + 1
            p = ppool.tile([C, cw], F32, tag="pf")
            nc.tensor.matmul(p[:], Ef[:], a[:], start=True, stop=True)
            an = apool.tile([C, cw], BF16, tag="af")
            # Pool (GPSIMD) multiply: no PSUM access-cycle penalty and no
            # post-exec drain, so the serial link is ~100ns shorter than DVE.
            nc.gpsimd.tensor_mul(an, p, ee_at(tf))
            a = an
            # backward step kk: matmul produces beta at t = T-2-kk; the
            # following multiply applies emission T-2-kk while that emission
            # still belongs to the backward half (t >= T/2+1).
            tb = T - 2 - kk
            if tb >= half + 1:
                p2 = ppool.tile([C, cw], F32, tag="pb")
                nc.tensor.matmul(p2[:], Eb[:], w[:], start=True, stop=True)
                wn = apool.tile([C, cw], BF16, tag="ab")
                nc.gpsimd.tensor_mul(wn, p2, ee_at(tb))
                w = wn
            elif tb == half:
                # final backward matmul yields beta_{T/2}; emission at T/2
                # belongs to the forward pass
                beta_ps = ppool.tile([C, cw], F32, tag="pb")
                nc.tensor.matmul(beta_ps[:], Eb[:], w[:], start=True, stop=True)
            if (kk + 1) % renorm == 0 and kk < half - 1:
                a = renorm_chain(a, off_f)
                w = renorm_chain(w, off_b)

        # Meet: logZ = log sum_j a[j]*beta[j] + offsets (+ MU*(T-1) on host).
        m = apool.tile([C, cw], BF16, tag="meet")
        nc.vector.tensor_mul(m, beta_ps, a)
        z = rpool.tile([1, cw], F32, tag="rs")
        nc.tensor.matmul(z[:], ones_col[:], m[:], start=True, stop=True)
        lg = small.tile([1, cw], F32, tag="lg")
        nc.scalar.activation(lg, z, AF.Ln)
        res = consts.tile([1, BLOC], F32, tag="res")
        nc.vector.tensor_add(res, lg, off_f)
        nc.vector.tensor_add(res, res, off_b)
        nc.sync.dma_start(out=out[:, :], in_=res[:])

    nc.compile()
    _cache[key] = nc
    return nc


def _gold_np(emissions, tags, mask, transitions, start_transitions, end_transitions):
    em = emissions.astype(np.float64)
    mf = mask.astype(np.float64)
    idx = np.arange(B)
    emit = np.take_along_axis(em, tags[:, :, None], axis=2)[:, :, 0]
    tr = transitions.astype(np.float64)[tags[:, :-1], tags[:, 1:]]
    score = start_transitions.astype(np.float64)[tags[:, 0]] + emit[:, 0]
    score = score + np.sum((emit[:, 1:] + tr) * mf[:, 1:], axis=1)
    last_idx = mask.astype(np.int64).sum(axis=1) - 1
    last_tags = tags[idx, last_idx]
    return score + end_transitions.astype(np.float64)[last_tags]


def _logz_host(emissions, mask, transitions, start_transitions, end_transitions):
    # Slow exact fallback (only for non-all-ones masks, which the spec never
    # produces).
    em = emissions.astype(np.float64)
    tr = transitions.astype(np.float64)
    alpha = start_transitions.astype(np.float64) + em[:, 0]
    for t in range(1, T):
        sc = alpha[:, :, None] + tr[None] + em[:, t, None, :]
        m = sc.max(axis=1)
        nxt = m + np.log(np.exp(sc - m[:, None, :]).sum(axis=1))
        alpha = np.where(mask[:, t, None], nxt, alpha)
    fin = alpha + end_transitions.astype(np.float64)[None]
    m = fin.max(axis=1)
    return m + np.log(np.exp(fin - m[:, None]).sum(axis=1))


def run_device(in_maps, trace=False, **kw):
    nc = _build()
    return bass_utils.run_bass_kernel_spmd(
        nc, in_maps, core_ids=list(range(NCORES)), trace=trace, **kw)


def make_in_maps(emissions, transitions, start_transitions, end_transitions):
    tr = np.ascontiguousarray(transitions, dtype=np.float32)
    trT = np.ascontiguousarray(transitions.T, dtype=np.float32)
    sv = np.ascontiguousarray(start_transitions, dtype=np.float32).reshape(C, 1)
    ev = np.ascontiguousarray(end_transitions, dtype=np.float32).reshape(C, 1)
    in_maps = []
    for k in range(NCORES):
        sl = slice(k * BLOC, (k + 1) * BLOC)
        em_k = np.ascontiguousarray(
            emissions[sl].transpose(2, 1, 0).astype(np.float32))
        in_maps.append({"em": em_k, "trans": tr, "transT": trT,
                        "startv": sv, "endv": ev})
    return in_maps


def kernel(**inputs):
    emissions = np.asarray(inputs["emissions"], dtype=np.float32)
    tags = np.asarray(inputs["tags"]).astype(np.int64)
    mask = np.asarray(inputs["mask"]).astype(bool)
    transitions = np.asarray(inputs["transitions"], dtype=np.float32)
    start_transitions = np.asarray(inputs["start_transitions"], dtype=np.float32)
    end_transitions = np.asarray(inputs["end_transitions"], dtype=np.float32)

    gold = _gold_np(emissions, tags, mask, transitions,
                    start_transitions, end_transitions)

    if mask.all():
        in_maps = make_in_maps(emissions, transitions,
                               start_transitions, end_transitions)
        res = run_device(in_maps)
        logz = np.concatenate([r["logz_out"][0] for r in res.results])
        # Eexp carries exp(-MU); it is applied on steps 1..T-1 only.
        logz = logz.astype(np.float64) + MU * (T - 1)
    else:
        logz = _logz_host(emissions, mask, transitions,
                          start_transitions, end_transitions)

    loss = np.mean(logz - gold)
    return np.asarray(loss, dtype=np.float32)



# revision 8
# speedup vs baseline: 1.2670x; 1.1086x over previous
"""Linear-chain CRF loss (mean over batch of logZ - gold_score) on 8 TRN2 cores.

Math: the forward (alpha) recursion runs in the exp domain so each step is a
single 128x128 @ 128x16 matmul on the PE plus one elementwise multiply:
    a_{t}[j,b] = ee_t[j,b] * sum_i E[i,j] * a_{t-1}[i,b]
with E = exp(transitions - MU) kept stationary (bf16 lhsT) and
ee_t = exp(emissions[b,t,:]) streamed from HBM in a host-pretransposed
(C, T, B_local) layout.  MU keeps per-step growth ~1; because the recursion
is linear, residual drift over 512 steps stays within a few log-units —
comfortably inside bf16 range — so no renormalization is needed.
Final: logz = log(alpha_half . beta_half) + MU*(T-1)  (host adds the MU term).

Bidirectional (meet-in-the-middle): forward runs t=1..T/2 while backward runs
t=T-1..T/2 concurrently, halving the serial chain to T/2 links.

Scheduling: raw Bass with manual semaphores (no Tile auto-sync).  Each hot-loop
instruction carries exactly ONE semaphore wait, so it pre-decodes into the
engine's wait queue instead of blocking the sequencer on an EventSemaphore.
The per-step multiply runs on the Pool/GPSIMD engine: unlike DVE it has no
PSUM access-cycle penalty and no post-execution drain, shortening the serial
PE->mul->PE link from ~530ns to ~420ns.

Sharding: data-parallel over batch, 16 sequences per core, no collectives;
host computes the (tiny) gold path score and the final mean.
"""

import numpy as np
from contextlib import ExitStack

import concourse.bass as bass
import concourse.bacc as bacc
import concourse.mybir as mybir
from concourse import bass_utils

B, T, C = 128, 1024, 128
NCORES = 8
BLOC = B // NCORES            # 16 sequences per core
TCH = 64                      # time steps per streamed emissions chunk
MU = 5.9                      # per-step log-growth pre-subtraction
RA = 4                        # state-tile ring depth per direction
NP = 3                        # PSUM-tile ring depth per direction

F32 = mybir.dt.float32
BF16 = mybir.dt.bfloat16
AF = mybir.ActivationFunctionType

_cache = {}


def _build():
    if "nc" in _cache:
        return _cache["nc"]
    half = T // 2
    nchunks = T // TCH
    # Stream order interleaves the two ends so the earliest-needed chunks of
    # each direction are first in queue order.
    order = []
    for i in range(nchunks // 2):
        order += [i, nchunks - 1 - i]
    pos = {ch: oi + 1 for oi, ch in enumerate(order)}

    nc = bacc.Bacc("TRN2", target_bir_lowering=False, debug=False)
    em = nc.dram_tensor("em", (C, T, BLOC), F32, kind="ExternalInput")
    trans = nc.dram_tensor("trans", (C, C), F32, kind="ExternalInput")
    transT = nc.dram_tensor("transT", (C, C), F32, kind="ExternalInput")
    startv = nc.dram_tensor("startv", (C, 1), F32, kind="ExternalInput")
    endv = nc.dram_tensor("endv", (C, 1), F32, kind="ExternalInput")
    out = nc.dram_tensor("logz_out", (1, BLOC), F32, kind="ExternalOutput")

    with ExitStack() as ctx:
        _n = iter(range(10**6))
        sbuf = lambda shape, dt: ctx.enter_context(
            nc.sbuf_tensor(f"sb{next(_n)}", shape, dt))
        psum = lambda shape, dt: ctx.enter_context(
            nc.psum_tensor(f"ps{next(_n)}", shape, dt))

        s_dma_c = nc.alloc_semaphore("s_dma_c")   # const DMAs (+16 each)
        s_dma_e = nc.alloc_semaphore("s_dma_e")   # emission chunk DMAs
        s_act = nc.alloc_semaphore("s_act")       # const exps on Act
        s_dve = nc.alloc_semaphore("s_dve")       # const scales on DVE
        s_exp = nc.alloc_semaphore("s_exp")       # emission chunk exps
        s_pef = nc.alloc_semaphore("s_pef")       # fwd matmuls done
        s_pof = nc.alloc_semaphore("s_pof")       # fwd muls done (init=1)
        s_peb = nc.alloc_semaphore("s_peb")       # bwd matmuls done
        s_pob = nc.alloc_semaphore("s_pob")       # bwd muls done (init=1)
        s_tail = nc.alloc_semaphore("s_tail")     # meet/finish chain
        s_out = nc.alloc_semaphore("s_out")       # output DMA

        trans_sb = sbuf([C, C], F32)
        transT_sb = sbuf([C, C], F32)
        sv = sbuf([C, 1], F32)
        ev = sbuf([C, 1], F32)
        Ef_f = sbuf([C, C], F32)
        Eb_f = sbuf([C, C], F32)
        Ef = sbuf([C, C], BF16)
        Eb = sbuf([C, C], BF16)
        Estart = sbuf([C, 1], F32)
        Eend = sbuf([C, 1], F32)
        ones_col = sbuf([C, 1], BF16)
        emraw = [sbuf([C, TCH, BLOC], F32) for _ in range(nchunks)]
        ee = [sbuf([C, TCH, BLOC], BF16) for _ in range(nchunks)]
        af = [sbuf([C, BLOC], BF16) for _ in range(RA)]
        ab = [sbuf([C, BLOC], BF16) for _ in range(RA)]
        mtile = sbuf([C, BLOC], BF16)
        lg = sbuf([1, BLOC], F32)

        pf = [psum([C, BLOC], F32) for _ in range(NP)]
        pb = [psum([C, BLOC], F32) for _ in range(NP)]
        beta_ps = psum([C, BLOC], F32)
        zps = psum([1, BLOC], F32)

        # --- SP queue: all input DMAs (consts first, chunks interleaved) ---
        nc.sync.dma_start(out=trans_sb[:, :], in_=trans[:, :]).then_inc(s_dma_c, 16)
        nc.sync.dma_start(out=transT_sb[:, :], in_=transT[:, :]).then_inc(s_dma_c, 16)
        nc.sync.dma_start(out=sv[:, :], in_=startv[:, :]).then_inc(s_dma_c, 16)
        nc.sync.dma_start(out=ev[:, :], in_=endv[:, :]).then_inc(s_dma_c, 16)
        for ch in order:
            nc.sync.dma_start(
                out=emraw[ch][:], in_=em[:, ch * TCH:(ch + 1) * TCH, :]
            ).then_inc(s_dma_e, 16)

        # --- Act queue: exps (consts, then chunks in stream order) ---
        nc.scalar.wait_ge(s_dma_c, 64)
        nc.scalar.activation(Ef_f[:], trans_sb[:], AF.Exp).then_inc(s_act, 1)
        nc.scalar.activation(Eb_f[:], transT_sb[:], AF.Exp).then_inc(s_act, 1)
        nc.scalar.activation(Estart[:], sv[:], AF.Exp).then_inc(s_act, 1)
        nc.scalar.activation(Eend[:], ev[:], AF.Exp).then_inc(s_act, 1)
        for oi, ch in enumerate(order):
            nc.scalar.wait_ge(s_dma_e, 16 * (oi + 1))
            nc.scalar.activation(ee[ch][:], emraw[ch][:], AF.Exp).then_inc(s_exp, 1)

        # --- DVE queue: fold exp(-MU) into stationary matrices; init states ---
        nc.vector.wait_ge(s_act, 2)
        nc.vector.tensor_scalar_mul(Ef[:], Ef_f[:], float(np.exp(-MU))).then_inc(s_dve, 1)
        nc.vector.tensor_scalar_mul(Eb[:], Eb_f[:], float(np.exp(-MU))).then_inc(s_dve, 1)
        nc.vector.memset(ones_col[:], 1.0).then_inc(s_dve, 1)
        # a_0 = ee_0 * exp(start); w_{T-1} = ee_{T-1} * exp(end)
        nc.vector.wait_ge(s_act, 4)
        nc.vector.wait_ge(s_exp, 2)  # chunks 0 and 15 (first two in order)
        nc.vector.tensor_scalar_mul(
            af[0][:], ee[0][:, 0, :], Estart[:, 0:1]).then_inc(s_pof, 1)
        nc.vector.tensor_scalar_mul(
            ab[0][:], ee[nchunks - 1][:, TCH - 1, :], Eend[:, 0:1]).then_inc(s_pob, 1)

        # --- PE prologue: weights ready before first ldweights ---
        nc.tensor.wait_ge(s_dve, 2)
        # --- Pool prologue: first chunks of both directions expd ---
        nc.gpsimd.wait_ge(s_exp, 2)

        # --- main loop: 512 iterations, two chains (fwd t=kk+1, bwd t=T-2-kk)
        # Chain invariants (in-order engines):
        #   mm_f(kk)  waits s_pof >= kk+1 (state kk ready), incs s_pef.
        #   TT_f(kk)  waits s_pef >= kk+1 (psum ready),     incs s_pof.
        # Ring WAR/WAW hazards are subsumed by these same-engine-ordered waits.
        for kk in range(half):
            tf = kk + 1
            tb = T - 2 - kk
            nc.tensor.matmul(
                pf[kk % NP][:], Ef[:], af[kk % RA][:], start=True, stop=True
            )._wait_ge(s_pof, kk + 1).then_inc(s_pef, 1)
            if tb >= half + 1:
                nc.tensor.matmul(
                    pb[kk % NP][:], Eb[:], ab[kk % RA][:], start=True, stop=True
                )._wait_ge(s_pob, kk + 1).then_inc(s_peb, 1)
            elif tb == half:
                # final backward matmul yields beta_{T/2}; emission at T/2
                # belongs to the forward pass
                nc.tensor.matmul(
                    beta_ps[:], Eb[:], ab[kk % RA][:], start=True, stop=True
                )._wait_ge(s_pob, kk + 1).then_inc(s_peb, 1)

            chf = tf // TCH
            if tf % TCH == 0:
                nc.gpsimd.wait_ge(s_exp, pos[chf])
            nc.gpsimd.tensor_mul(
                af[(kk + 1) % RA][:], pf[kk % NP][:], ee[chf][:, tf % TCH, :]
            )._wait_ge(s_pef, kk + 1).then_inc(s_pof, 1)
            if tb >= half + 1:
                chb = tb // TCH
                if tb % TCH == TCH - 1:
                    nc.gpsimd.wait_ge(s_exp, pos[chb])
                nc.gpsimd.tensor_mul(
                    ab[(kk + 1) % RA][:], pb[kk % NP][:], ee[chb][:, tb % TCH, :]
                )._wait_ge(s_peb, kk + 1).then_inc(s_pob, 1)

        # --- meet: logZ = log sum_j alpha_half[j] * beta_half[j] ---
        nc.gpsimd.wait_ge(s_pof, half + 1)          # TT_f(511) done
        nc.gpsimd.tensor_mul(
            mtile[:], beta_ps[:], af[half % RA][:]
        )._wait_ge(s_peb, half - 1).then_inc(s_tail, 1)
        nc.tensor.wait_ge(s_dve, 3)                 # ones_col ready
        nc.tensor.matmul(
            zps[:], ones_col[:], mtile[:], start=True, stop=True
        )._wait_ge(s_tail, 1).then_inc(s_tail, 1)
        nc.scalar.activation(lg[:], zps[:], AF.Ln)._wait_ge(s_tail, 2).then_inc(s_tail, 1)
        nc.sync.dma_start(out=out[:, :], in_=lg[:])._wait_ge(s_tail, 3).then_inc(s_out, 16)
        nc.sync.wait_ge(s_out, 16)

    nc.compile()
    _cache["nc"] = nc
    return nc


def _gold_np(emissions, tags, mask, transitions, start_transitions, end_transitions):
    em = emissions.astype(np.float64)
    mf = mask.astype(np.float64)
    idx = np.arange(B)
    emit = np.take_along_axis(em, tags[:, :, None], axis=2)[:, :, 0]
    tr = transitions.astype(np.float64)[tags[:, :-1], tags[:, 1:]]
    score = start_transitions.astype(np.float64)[tags[:, 0]] + emit[:, 0]
    score = score + np.sum((emit[:, 1:] + tr) * mf[:, 1:], axis=1)
    last_idx = mask.astype(np.int64).sum(axis=1) - 1
    last_tags = tags[idx, last_idx]
    return score + end_transitions.astype(np.float64)[last_tags]


def _logz_host(emissions, mask, transitions, start_transitions, end_transitions):
    # Slow exact fallback (only for non-all-ones masks, which the spec never
    # produces).
    em = emissions.astype(np.float64)
    tr = transitions.astype(np.float64)
    alpha = start_transitions.astype(np.float64) + em[:, 0]
    for t in range(1, T):
        sc = alpha[:, :, None] + tr[None] + em[:, t, None, :]
        m = sc.max(axis=1)
        nxt = m + np.log(np.exp(sc - m[:, None, :]).sum(axis=1))
        alpha = np.where(mask[:, t, None], nxt, alpha)
    fin = alpha + end_transitions.astype(np.float64)[None]
    m = fin.max(axis=1)
    return m + np.log(np.exp(fin - m[:, None]).sum(axis=1))


def run_device(in_maps, trace=False, **kw):
    nc = _build()
    return bass_utils.run_bass_kernel_spmd(
        nc, in_maps, core_ids=list(range(NCORES)), trace=trace, **kw)


def make_in_maps(emissions, transitions, start_transitions, end_transitions):
    tr = np.ascontiguousarray(transitions, dtype=np.float32)
    trT = np.ascontiguousarray(transitions.T, dtype=np.float32)
    sv = np.ascontiguousarray(start_transitions, dtype=np.float32).reshape(C, 1)
    ev = np.ascontiguousarray(end_transitions, dtype=np.float32).reshape(C, 1)
    in_maps = []
    for k in range(NCORES):
        sl = slice(k * BLOC, (k + 1) * BLOC)
        em_k = np.ascontiguousarray(
            emissions[sl].transpose(2, 1, 0).astype(np.float32))
        in_maps.append({"em": em_k, "trans": tr, "transT": trT,
                        "startv": sv, "endv": ev})
    return in_maps


def kernel(**inputs):
    emissions = np.asarray(inputs["emissions"], dtype=np.float32)
    tags = np.asarray(inputs["tags"]).astype(np.int64)
    mask = np.asarray(inputs["mask"]).astype(bool)
    transitions = np.asarray(inputs["transitions"], dtype=np.float32)
    start_transitions = np.asarray(inputs["start_transitions"], dtype=np.float32)
    end_transitions = np.asarray(inputs["end_transitions"], dtype=np.float32)

    gold = _gold_np(emissions, tags, mask, transitions,
                    start_transitions, end_transitions)

    if mask.all():
        in_maps = make_in_maps(emissions, transitions,
                               start_transitions, end_transitions)
        res = run_device(in_maps)
        logz = np.concatenate([r["logz_out"][0] for r in res.results])
        # E carries exp(-MU); it is applied on steps 1..T-1 only.
        logz = logz.astype(np.float64) + MU * (T - 1)
    else:
        logz = _logz_host(emissions, mask, transitions,
                          start_transitions, end_transitions)

    loss = np.mean(logz - gold)
    return np.asarray(loss, dtype=np.float32)


# revision 10
# speedup vs baseline: 5.8240x; 4.5967x over previous
"""Linear-chain CRF loss (mean over batch of logZ - gold_score) on 8 TRN2 cores.

Math: the forward (alpha) recursion runs in the exp domain:
    a_t = ee_t * (E^T a_{t-1}),   E = exp(transitions - MU),  ee = exp(emissions)
logZ = log(eend^T a_{T-1}) + MU*(T-1).

Key restructure — segmented scan with warmup: products of positive matrices
forget their initial direction at the Birkhoff contraction rate, measured here
at ~0.15x per step (Hilbert distance 2e-10 after 12 steps).  So the serial
T-1 = 1023-step chain is split into S=32 segments; each segment's start
direction u_s is recovered by warming up W~12 steps from a uniform vector
using the true preceding emissions.  The exact telescoping identity
    logZ = sum_s log(n_s^T P_s u_s) - sum_{s>=1} log(1^T u_s)
(n_s = ones except eend for the last segment; segment 0 starts from the exact
a_0) makes the answer independent of the u_s scale; direction error enters
once per boundary and is ~1e-10, far below the bf16 noise floor.

All 32 chains run concurrently in lockstep bundles of G=16 chains: per
iteration each bundle does ONE [128x128]@[128x256] matmul (PE) and ONE
[128,256] elementwise multiply (DVE), amortizing the fixed PSUM-access cost
over 256 columns.  This converts the latency-bound serial recursion
(~535ns/step) into a throughput-bound pipeline (~32ns/step/chain).

Scheduling: raw Bass with manual semaphores (one wait per instruction, so
instructions pre-decode into wait queues instead of blocking sequencers).
The per-step multiply must run on DVE: GPSIMD/Pool cannot access PSUM, and
the Activation engine only supports per-partition scalar operands.

Sharding: data-parallel over batch, 16 sequences per core, no collectives;
host computes the (tiny) gold path score, capture logs, and the final mean.
"""

import numpy as np
from contextlib import ExitStack

import ml_dtypes
import concourse.bass as bass
import concourse.bacc as bacc
import concourse.mybir as mybir
from concourse import bass_utils

B, T, C = 128, 1024, 128
NCORES = 8
BLOC = B // NCORES            # 16 sequences per core
S = 32                        # time segments (chains) per core
G = 16                        # chains per lockstep bundle
NB = S // G                   # bundles
CW = G * BLOC                 # bundle width (matmul/mul free dim) = 256
NL = 44                       # lockstep iterations per chain
# Segment lengths: chain 0 starts from the exact a_0 (no warmup); the rest
# warm up for NL - L_k steps (12 or 13).  sum(LS) = T-1 = 1023.
LS = [44] + [32] * 18 + [31] * 13
WS = [NL - L for L in LS]     # warmup iterations per chain
CHSLOT = 5                    # emission slots per streamed chunk
NCHUNK = (NL + 1) // CHSLOT   # 45 slots = 9 chunks of 5
MU = 5.9                      # per-step log-growth pre-subtraction
RA = 4                        # state-tile ring depth per bundle
NP = 3                        # PSUM-tile ring depth per bundle

F32 = mybir.dt.float32
BF16 = mybir.dt.bfloat16
AF = mybir.ActivationFunctionType

_cache = {}


def _plan():
    """Per-chain emission stream: ts[k][j] = original t for slot j (j=0 is
    the init slot, only used by chain 0 for a_0 = ee_0 * exp(start))."""
    bs = np.concatenate([[1], 1 + np.cumsum(LS)])[:S]  # segment starts
    ts = np.zeros((S, NL + 1), dtype=np.int64)
    for k in range(S):
        first = bs[k] - WS[k]           # first warmup step (chain 0: 1)
        ts[k, 1:] = first + np.arange(NL)
        ts[k, 0] = 0 if k == 0 else first
    return ts


def _build():
    if "nc" in _cache:
        return _cache["nc"]
    nc = bacc.Bacc("TRN2", target_bir_lowering=False, debug=False)
    em = nc.dram_tensor("em", (C, NL + 1, S * BLOC), BF16, kind="ExternalInput")
    trans = nc.dram_tensor("trans", (C, C), F32, kind="ExternalInput")
    startv = nc.dram_tensor("startv", (C, 1), F32, kind="ExternalInput")
    endv = nc.dram_tensor("endv", (C, 1), F32, kind="ExternalInput")
    caps = nc.dram_tensor("caps", (2, S * BLOC), F32, kind="ExternalOutput")

    with ExitStack() as ctx:
        _n = iter(range(10 ** 6))
        sbuf = lambda shape, dt: ctx.enter_context(
            nc.sbuf_tensor(f"sb{next(_n)}", shape, dt))
        psum = lambda shape, dt: ctx.enter_context(
            nc.psum_tensor(f"ps{next(_n)}", shape, dt))

        s_dma_c = nc.alloc_semaphore("s_dma_c")   # const DMAs (+16 each)
        s_dma_e = nc.alloc_semaphore("s_dma_e")   # emission chunk DMAs
        s_act = nc.alloc_semaphore("s_act")       # const exps on Act
        s_dve = nc.alloc_semaphore("s_dve")       # const prep on DVE
        s_exp = nc.alloc_semaphore("s_exp")       # emission chunk exps
        s_pe = [nc.alloc_semaphore(f"s_pe{g}") for g in range(NB)]
        s_po = [nc.alloc_semaphore(f"s_po{g}") for g in range(NB)]
        s_cap = nc.alloc_semaphore("s_cap")       # capture matmuls
        s_tail = nc.alloc_semaphore("s_tail")
        s_out = nc.alloc_semaphore("s_out")

        trans_sb = sbuf([C, C], F32)
        sv = sbuf([C, 1], F32)
        ev = sbuf([C, 1], F32)
        Ef_f = sbuf([C, C], F32)
        Ef = sbuf([C, C], BF16)
        Estart = sbuf([C, 1], F32)
        Eend_f = sbuf([C, 1], F32)
        Eend = sbuf([C, 1], BF16)
        ones_col = sbuf([C, 1], BF16)
        emraw = [sbuf([C, CHSLOT, S * BLOC], BF16) for _ in range(NCHUNK)]
        ee = [sbuf([C, CHSLOT, S * BLOC], BF16) for _ in range(NCHUNK)]
        st = [[sbuf([C, CW], BF16) for _ in range(RA)] for _ in range(NB)]
        sc_sb = sbuf([1, S * BLOC], F32)
        ec_sb = sbuf([1, S * BLOC], F32)

        ps = [[psum([C, CW], F32) for _ in range(NP)] for _ in range(NB)]
        scap = psum([1, S * BLOC], F32)
        ecap = psum([1, S * BLOC], F32)

        def ee_sl(g, slot):
            return ee[slot // CHSLOT][:, slot % CHSLOT, g * CW:(g + 1) * CW]

        # --- SP queue: input DMAs ---
        nc.sync.dma_start(out=trans_sb[:, :], in_=trans[:, :]).then_inc(s_dma_c, 16)
        nc.sync.dma_start(out=sv[:, :], in_=startv[:, :]).then_inc(s_dma_c, 16)
        nc.sync.dma_start(out=ev[:, :], in_=endv[:, :]).then_inc(s_dma_c, 16)
        for ci in range(NCHUNK):
            nc.sync.dma_start(
                out=emraw[ci][:], in_=em[:, ci * CHSLOT:(ci + 1) * CHSLOT, :]
            ).then_inc(s_dma_e, 16)

        # --- Act queue: exps ---
        nc.scalar.wait_ge(s_dma_c, 48)
        nc.scalar.activation(Ef_f[:], trans_sb[:], AF.Exp).then_inc(s_act, 1)
        nc.scalar.activation(Estart[:], sv[:], AF.Exp).then_inc(s_act, 1)
        nc.scalar.activation(Eend_f[:], ev[:], AF.Exp).then_inc(s_act, 1)
        for ci in range(NCHUNK):
            nc.scalar.wait_ge(s_dma_e, 16 * (ci + 1))
            nc.scalar.activation(ee[ci][:], emraw[ci][:], AF.Exp).then_inc(s_exp, 1)

        # --- DVE queue: const prep + state init ---
        nc.vector.wait_ge(s_act, 1)
        nc.vector.tensor_scalar_mul(Ef[:], Ef_f[:], float(np.exp(-MU))).then_inc(s_dve, 1)
        nc.vector.memset(ones_col[:], 1.0).then_inc(s_dve, 1)
        nc.vector.wait_ge(s_act, 3)
        nc.vector.tensor_scalar_mul(Eend[:], Eend_f[:], 1.0).then_inc(s_dve, 1)
        nc.vector.memset(st[0][0][:], 1.0)
        nc.vector.memset(st[1][0][:], 1.0).then_inc(s_po[1], 1)
        nc.vector.wait_ge(s_exp, 1)
        # exact a_0 for chain 0 (bundle 0, slot 0 of the init emission slot)
        nc.vector.tensor_scalar_mul(
            st[0][0][:, 0:BLOC], ee[0][:, 0, 0:BLOC], Estart[:, 0:1]
        ).then_inc(s_po[0], 1)

        # --- PE prologue ---
        nc.tensor.wait_ge(s_dve, 3)

        cap_at = {}
        for k in range(S):
            cap_at.setdefault(WS[k], []).append(k)

        # --- main loop: NL lockstep iterations over NB bundles ---
        # x_i lives in st[g][i % RA]; TT(i) maps x_i -> x_{i+1}.
        #   mm(g,i)  waits s_po[g] >= i+1, incs s_pe[g]
        #   TT(g,i)  waits s_pe[g] >= i+1, incs s_po[g]
        for i in range(NL):
            slot = i + 1
            if slot % CHSLOT == 0:
                nc.vector.wait_ge(s_exp, slot // CHSLOT + 1)
            for g in range(NB):
                nc.tensor.matmul(
                    ps[g][i % NP][:], Ef[:], st[g][i % RA][:], start=True, stop=True
                )._wait_ge(s_po[g], i + 1).then_inc(s_pe[g], 1)
            # start-captures: 1^T u_k for chains whose warmup ends here
            for k in cap_at.get(i, []):
                g, c = divmod(k, G)
                nc.tensor.matmul(
                    scap[:, k * BLOC:(k + 1) * BLOC], ones_col[:],
                    st[g][i % RA][:, c * BLOC:(c + 1) * BLOC], start=True, stop=True
                )._wait_ge(s_po[g], i + 1).then_inc(s_cap, 1)
            for g in range(NB):
                nc.vector.tensor_mul(
                    st[g][(i + 1) % RA][:], ps[g][i % NP][:], ee_sl(g, slot)
                )._wait_ge(s_pe[g], i + 1).then_inc(s_po[g], 1)

        # --- end-captures: n_k^T x_NL (eend for the last chain) ---
        for k in range(S):
            g, c = divmod(k, G)
            lhs = Eend if k == S - 1 else ones_col
            nc.tensor.matmul(
                ecap[:, k * BLOC:(k + 1) * BLOC], lhs[:],
                st[g][NL % RA][:, c * BLOC:(c + 1) * BLOC], start=True, stop=True
            )._wait_ge(s_po[g], NL + 1).then_inc(s_cap, 1)

        # --- tail: copy captures to SBUF, DMA out ---
        nc.scalar.activation(sc_sb[:], scap[:], AF.Copy)._wait_ge(s_cap, 2 * S).then_inc(s_tail, 1)
        nc.scalar.activation(ec_sb[:], ecap[:], AF.Copy).then_inc(s_tail, 1)
        nc.sync.wait_ge(s_tail, 2)
        nc.sync.dma_start(out=caps[0:1, :], in_=sc_sb[:]).then_inc(s_out, 16)
        nc.sync.dma_start(out=caps[1:2, :], in_=ec_sb[:]).then_inc(s_out, 16)
        nc.sync.wait_ge(s_out, 32)

    nc.compile()
    _cache["nc"] = nc
    return nc


def _gold_np(emissions, tags, mask, transitions, start_transitions, end_transitions):
    em = emissions.astype(np.float64)
    mf = mask.astype(np.float64)
    idx = np.arange(B)
    emit = np.take_along_axis(em, tags[:, :, None], axis=2)[:, :, 0]
    tr = transitions.astype(np.float64)[tags[:, :-1], tags[:, 1:]]
    score = start_transitions.astype(np.float64)[tags[:, 0]] + emit[:, 0]
    score = score + np.sum((emit[:, 1:] + tr) * mf[:, 1:], axis=1)
    last_idx = mask.astype(np.int64).sum(axis=1) - 1
    last_tags = tags[idx, last_idx]
    return score + end_transitions.astype(np.float64)[last_tags]


def _logz_host(emissions, mask, transitions, start_transitions, end_transitions):
    # Slow exact fallback (only for non-all-ones masks, which the spec never
    # produces).
    em = emissions.astype(np.float64)
    tr = transitions.astype(np.float64)
    alpha = start_transitions.astype(np.float64) + em[:, 0]
    for t in range(1, T):
        sc = alpha[:, :, None] + tr[None] + em[:, t, None, :]
        m = sc.max(axis=1)
        nxt = m + np.log(np.exp(sc - m[:, None, :]).sum(axis=1))
        alpha = np.where(mask[:, t, None], nxt, alpha)
    fin = alpha + end_transitions.astype(np.float64)[None]
    m = fin.max(axis=1)
    return m + np.log(np.exp(fin - m[:, None]).sum(axis=1))


def run_device(in_maps, trace=False, **kw):
    nc = _build()
    return bass_utils.run_bass_kernel_spmd(
        nc, in_maps, core_ids=list(range(NCORES)), trace=trace, **kw)


def make_in_maps(emissions, transitions, start_transitions, end_transitions):
    ts = _plan()
    tr = np.ascontiguousarray(transitions, dtype=np.float32)
    sv = np.ascontiguousarray(start_transitions, dtype=np.float32).reshape(C, 1)
    ev = np.ascontiguousarray(end_transitions, dtype=np.float32).reshape(C, 1)
    in_maps = []
    for k in range(NCORES):
        sl = slice(k * BLOC, (k + 1) * BLOC)
        em_k = emissions[sl].transpose(2, 1, 0)          # (C, T, BLOC)
        # (C, S, NL+1, BLOC) -> (C, NL+1, S, BLOC) -> (C, NL+1, S*BLOC)
        em_g = em_k[:, ts, :].transpose(0, 2, 1, 3).reshape(C, NL + 1, S * BLOC)
        em_g = np.ascontiguousarray(em_g.astype(ml_dtypes.bfloat16))
        in_maps.append({"em": em_g, "trans": tr, "startv": sv, "endv": ev})
    return in_maps


def kernel(**inputs):
    emissions = np.asarray(inputs["emissions"], dtype=np.float32)
    tags = np.asarray(inputs["tags"]).astype(np.int64)
    mask = np.asarray(inputs["mask"]).astype(bool)
    transitions = np.asarray(inputs["transitions"], dtype=np.float32)
    start_transitions = np.asarray(inputs["start_transitions"], dtype=np.float32)
    end_transitions = np.asarray(inputs["end_transitions"], dtype=np.float32)

    gold = _gold_np(emissions, tags, mask, transitions,
                    start_transitions, end_transitions)

    if mask.all():
        in_maps = make_in_maps(emissions, transitions,
                               start_transitions, end_transitions)
        res = run_device(in_maps)
        logz = np.empty(B, dtype=np.float64)
        for k in range(NCORES):
            caps = np.asarray(res.results[k]["caps"], dtype=np.float64)
            sc = caps[0].reshape(S, BLOC)
            ec = caps[1].reshape(S, BLOC)
            # telescoped segment growths; E carries exp(-MU) on each of the
            # 1023 real steps
            lz = np.log(ec).sum(0) - np.log(sc[1:]).sum(0) + MU * (T - 1)
            logz[k * BLOC:(k + 1) * BLOC] = lz
    else:
        logz = _logz_host(emissions, mask, transitions,
                          start_transitions, end_transitions)

    loss = np.mean(logz - gold)
    return np.asarray(loss, dtype=np.float32)


# revision 16
# speedup vs baseline: 6.5771x; 1.1293x over previous
"""Linear-chain CRF loss (mean over batch of logZ - gold_score) on 8 TRN2 cores.

Math: the forward (alpha) recursion runs in the exp domain:
    a_t = ee_t * (E^T a_{t-1}),   E = exp(transitions - MU),  ee = exp(emissions)
logZ = log(eend^T a_{T-1}) + MU*(T-1).

Key restructure — segmented scan with warmup: products of positive matrices
forget their initial direction at the Birkhoff contraction rate, measured here
at ~0.15x per step (Hilbert distance 2e-10 after 12 steps).  So the serial
T-1 = 1023-step chain is split into S=32 segments; each segment's start
direction u_s is recovered by warming up W~12 steps from a uniform vector
using the true preceding emissions.  The exact telescoping identity
    logZ = sum_s log(n_s^T P_s u_s) - sum_{s>=1} log(1^T u_s)
(n_s = ones except eend for the last segment; segment 0 starts from the exact
a_0) makes the answer independent of the u_s scale; direction error enters
once per boundary and is ~1e-10, far below the bf16 noise floor.

All 32 chains run concurrently in lockstep bundles of G=16 chains: per
iteration each bundle does ONE [128x128]@[128x256] matmul (PE) and ONE
[128,256] elementwise multiply (DVE), amortizing the fixed PSUM-access cost
over 256 columns.  This converts the latency-bound serial recursion
(~535ns/step) into a throughput-bound pipeline (~32ns/step/chain).

Scheduling: raw Bass with manual semaphores (one wait per instruction, so
instructions pre-decode into wait queues instead of blocking sequencers).
The per-step multiply must run on DVE: GPSIMD/Pool cannot access PSUM, and
the Activation engine only supports per-partition scalar operands.

Sharding: data-parallel over batch, 16 sequences per core, no collectives;
host computes the (tiny) gold path score, capture logs, and the final mean.
"""

import numpy as np
from contextlib import ExitStack

import ml_dtypes
import concourse.bass as bass
import concourse.bacc as bacc
import concourse.mybir as mybir
from concourse import bass_utils

B, T, C = 128, 1024, 128
NCORES = 8
BLOC = B // NCORES            # 16 sequences per core
S = 32                        # time segments (chains) per core
G = 16                        # chains per lockstep bundle
NB = S // G                   # bundles
CW = G * BLOC                 # bundle width (matmul/mul free dim) = 256
NL = 40                       # lockstep iterations per chain
# Segment lengths: chain 0 starts from the exact a_0 (no warmup); the rest
# warm up for NL - L_k steps (8 or 9).  sum(LS) = T-1 = 1023.
LS = [40] + [32] * 22 + [31] * 9
WS = [NL - L for L in LS]     # warmup iterations per chain
# Emission chunk sizes in slots (NL+1 = 41 total); a small first chunk gets
# the init slot expd quickly so the pipeline starts early.
CS = [2] + [5] * 7 + [4]
NCHUNK = len(CS)
_CHUNK_OF = []                # slot -> chunk index
_CHUNK_BASE = []              # chunk -> first slot
for _ci, _n in enumerate(CS):
    _CHUNK_BASE.append(len(_CHUNK_OF))
    _CHUNK_OF += [_ci] * _n
MU = 5.9                      # per-step log-growth pre-subtraction
RA = 4                        # state-tile ring depth per bundle
NP = 3                        # PSUM-tile ring depth per bundle

F32 = mybir.dt.float32
BF16 = mybir.dt.bfloat16
AF = mybir.ActivationFunctionType

_cache = {}


def _plan():
    """Per-chain emission stream: ts[k][j] = original t for slot j (j=0 is
    the init slot, only used by chain 0 for a_0 = ee_0 * exp(start))."""
    bs = np.concatenate([[1], 1 + np.cumsum(LS)])[:S]  # segment starts
    ts = np.zeros((S, NL + 1), dtype=np.int64)
    for k in range(S):
        first = bs[k] - WS[k]           # first warmup step (chain 0: 1)
        ts[k, 1:] = first + np.arange(NL)
        ts[k, 0] = 0 if k == 0 else first
    return ts


def _build():
    if "nc" in _cache:
        return _cache["nc"]
    nc = bacc.Bacc("TRN2", target_bir_lowering=False, debug=False)
    em = nc.dram_tensor("em", (C, NL + 1, S * BLOC), BF16, kind="ExternalInput")
    trans = nc.dram_tensor("trans", (C, C), F32, kind="ExternalInput")
    startv = nc.dram_tensor("startv", (C, 1), F32, kind="ExternalInput")
    endv = nc.dram_tensor("endv", (C, 1), F32, kind="ExternalInput")
    caps = nc.dram_tensor("caps", (2, S * BLOC), F32, kind="ExternalOutput")

    with ExitStack() as ctx:
        _n = iter(range(10 ** 6))
        sbuf = lambda shape, dt: ctx.enter_context(
            nc.sbuf_tensor(f"sb{next(_n)}", shape, dt))
        psum = lambda shape, dt: ctx.enter_context(
            nc.psum_tensor(f"ps{next(_n)}", shape, dt))

        s_dma_c = nc.alloc_semaphore("s_dma_c")   # const DMAs (+16 each)
        s_dma_e = nc.alloc_semaphore("s_dma_e")   # emission chunk DMAs
        s_act = nc.alloc_semaphore("s_act")       # const exps on Act
        s_dve = nc.alloc_semaphore("s_dve")       # const prep on DVE
        s_exp = nc.alloc_semaphore("s_exp")       # emission chunk exps
        s_pe = [nc.alloc_semaphore(f"s_pe{g}") for g in range(NB)]
        s_po = [nc.alloc_semaphore(f"s_po{g}") for g in range(NB)]
        s_cap = nc.alloc_semaphore("s_cap")       # capture matmuls
        s_tail = nc.alloc_semaphore("s_tail")
        s_out = nc.alloc_semaphore("s_out")

        trans_sb = sbuf([C, C], F32)
        sv = sbuf([C, 1], F32)
        ev = sbuf([C, 1], F32)
        Ef_f = sbuf([C, C], F32)
        Ef = sbuf([C, C], BF16)
        Estart = sbuf([C, 1], F32)
        Eend_f = sbuf([C, 1], F32)
        Eend = sbuf([C, 1], BF16)
        ones_col = sbuf([C, 1], BF16)
        warm = sbuf([1, 1], F32)
        emraw = [sbuf([C, CS[ci], S * BLOC], BF16) for ci in range(NCHUNK)]
        ee = [sbuf([C, CS[ci], S * BLOC], BF16) for ci in range(NCHUNK)]
        st = [[sbuf([C, CW], BF16) for _ in range(RA)] for _ in range(NB)]
        sc_sb = sbuf([1, S * BLOC], F32)
        ec_sb = sbuf([1, S * BLOC], F32)

        ps = [[psum([C, CW], F32) for _ in range(NP)] for _ in range(NB)]
        scap = psum([1, S * BLOC], F32)
        ecap = psum([1, S * BLOC], F32)

        def ee_sl(g, slot):
            ci = _CHUNK_OF[slot]
            return ee[ci][:, slot - _CHUNK_BASE[ci], g * CW:(g + 1) * CW]

        # --- SP queue: input DMAs (first emission chunk before consts) ---
        nc.sync.dma_start(
            out=emraw[0][:], in_=em[:, 0:CS[0], :]).then_inc(s_dma_e, 16)
        nc.sync.dma_start(out=trans_sb[:, :], in_=trans[:, :]).then_inc(s_dma_c, 16)
        nc.sync.dma_start(out=sv[:, :], in_=startv[:, :]).then_inc(s_dma_c, 16)
        nc.sync.dma_start(out=ev[:, :], in_=endv[:, :]).then_inc(s_dma_c, 16)
        for ci in range(1, NCHUNK):
            base = _CHUNK_BASE[ci]
            nc.sync.dma_start(
                out=emraw[ci][:], in_=em[:, base:base + CS[ci], :]
            ).then_inc(s_dma_e, 16)

        # --- Act queue: exps (dummy first so the Exp table loads during DMA) ---
        nc.scalar.activation(warm[:], warm[:], AF.Exp)
        nc.scalar.wait_ge(s_dma_c, 48)
        nc.scalar.activation(Ef_f[:], trans_sb[:], AF.Exp).then_inc(s_act, 1)
        nc.scalar.activation(Estart[:], sv[:], AF.Exp).then_inc(s_act, 1)
        nc.scalar.activation(Eend_f[:], ev[:], AF.Exp).then_inc(s_act, 1)
        for ci in range(NCHUNK):
            nc.scalar.wait_ge(s_dma_e, 16 * (ci + 1))
            nc.scalar.activation(ee[ci][:], emraw[ci][:], AF.Exp).then_inc(s_exp, 1)

        # --- DVE queue: const prep + state init ---
        nc.vector.wait_ge(s_act, 1)
        nc.vector.tensor_scalar_mul(Ef[:], Ef_f[:], float(np.exp(-MU))).then_inc(s_dve, 1)
        nc.vector.memset(ones_col[:], 1.0).then_inc(s_dve, 1)
        nc.vector.wait_ge(s_act, 3)
        nc.vector.tensor_scalar_mul(Eend[:], Eend_f[:], 1.0).then_inc(s_dve, 1)
        nc.vector.memset(st[0][0][:], 1.0)
        nc.vector.memset(st[1][0][:], 1.0).then_inc(s_po[1], 1)
        nc.vector.wait_ge(s_exp, 1)
        # exact a_0 for chain 0 (bundle 0, slot 0 of the init emission slot)
        nc.vector.tensor_scalar_mul(
            st[0][0][:, 0:BLOC], ee[0][:, 0, 0:BLOC], Estart[:, 0:1]
        ).then_inc(s_po[0], 1)

        # --- PE prologue ---
        nc.tensor.wait_ge(s_dve, 3)

        def runs_of(ks):
            """Group chain ids into (bundle, c_lo, c_hi) runs of adjacent
            columns so each capture is one wide matmul."""
            out = []
            for k in sorted(ks):
                g, c = divmod(k, G)
                if out and out[-1][0] == g and out[-1][2] == c - 1:
                    out[-1][2] = c
                else:
                    out.append([g, c, c])
            return out

        cap_at = {}
        for k in range(S):
            cap_at.setdefault(WS[k], []).append(k)
        ncap = 0

        # --- main loop: NL lockstep iterations over NB bundles ---
        # x_i lives in st[g][i % RA]; TT(i) maps x_i -> x_{i+1}.
        #   mm(g,i)  waits s_po[g] >= i+1, incs s_pe[g]
        #   TT(g,i)  waits s_pe[g] >= i+1, incs s_po[g]
        for i in range(NL):
            slot = i + 1
            if _CHUNK_OF[slot] != _CHUNK_OF[slot - 1]:
                nc.vector.wait_ge(s_exp, _CHUNK_OF[slot] + 1)
            for g in range(NB):
                nc.tensor.matmul(
                    ps[g][i % NP][:], Ef[:], st[g][i % RA][:], start=True, stop=True
                )._wait_ge(s_po[g], i + 1).then_inc(s_pe[g], 1)
            # start-captures: 1^T u_k for chains whose warmup ends here
            for g, c0, c1 in runs_of(cap_at.get(i, [])):
                nc.tensor.matmul(
                    scap[:, (g * G + c0) * BLOC:(g * G + c1 + 1) * BLOC],
                    ones_col[:],
                    st[g][i % RA][:, c0 * BLOC:(c1 + 1) * BLOC],
                    start=True, stop=True,
                )._wait_ge(s_po[g], i + 1).then_inc(s_cap, 1)
                ncap += 1
            for g in range(NB):
                nc.vector.tensor_mul(
                    st[g][(i + 1) % RA][:], ps[g][i % NP][:], ee_sl(g, slot)
                )._wait_ge(s_pe[g], i + 1).then_inc(s_po[g], 1)

        nscap = ncap  # start-captures emitted so far

        # --- end-captures: n_k^T x_NL (eend weighting for the last chain) ---
        for g, c0, c1 in runs_of(range(S - 1)):
            nc.tensor.matmul(
                ecap[:, (g * G + c0) * BLOC:(g * G + c1 + 1) * BLOC],
                ones_col[:],
                st[g][NL % RA][:, c0 * BLOC:(c1 + 1) * BLOC],
                start=True, stop=True,
            )._wait_ge(s_po[g], NL + 1).then_inc(s_cap, 1)
            ncap += 1
        nc.tensor.matmul(
            ecap[:, (S - 1) * BLOC:S * BLOC], Eend[:],
            st[NB - 1][NL % RA][:, (G - 1) * BLOC:G * BLOC],
            start=True, stop=True,
        )._wait_ge(s_po[NB - 1], NL + 1).then_inc(s_cap, 1)
        ncap += 1

        # --- tail: copy captures PSUM->SBUF on Act (DMA can't read PSUM),
        # start-captures copied as soon as they're all done (mid-loop) ---
        nc.scalar.activation(
            sc_sb[:], scap[:], AF.Copy)._wait_ge(s_cap, nscap).then_inc(s_tail, 1)
        nc.scalar.activation(
            ec_sb[:], ecap[:], AF.Copy)._wait_ge(s_cap, ncap).then_inc(s_tail, 1)
        nc.sync.wait_ge(s_tail, 1)
        nc.sync.dma_start(out=caps[0:1, :], in_=sc_sb[:]).then_inc(s_out, 16)
        nc.sync.wait_ge(s_tail, 2)
        nc.sync.dma_start(out=caps[1:2, :], in_=ec_sb[:]).then_inc(s_out, 16)

    nc.compile()
    _cache["nc"] = nc
    return nc


def _gold_np(emissions, tags, mask, transitions, start_transitions, end_transitions):
    em = emissions.astype(np.float64)
    mf = mask.astype(np.float64)
    idx = np.arange(B)
    emit = np.take_along_axis(em, tags[:, :, None], axis=2)[:, :, 0]
    tr = transitions.astype(np.float64)[tags[:, :-1], tags[:, 1:]]
    score = start_transitions.astype(np.float64)[tags[:, 0]] + emit[:, 0]
    score = score + np.sum((emit[:, 1:] + tr) * mf[:, 1:], axis=1)
    last_idx = mask.astype(np.int64).sum(axis=1) - 1
    last_tags = tags[idx, last_idx]
    return score + end_transitions.astype(np.float64)[last_tags]


def _logz_host(emissions, mask, transitions, start_transitions, end_transitions):
    # Slow exact fallback (only for non-all-ones masks, which the spec never
    # produces).
    em = emissions.astype(np.float64)
    tr = transitions.astype(np.float64)
    alpha = start_transitions.astype(np.float64) + em[:, 0]
    for t in range(1, T):
        sc = alpha[:, :, None] + tr[None] + em[:, t, None, :]
        m = sc.max(axis=1)
        nxt = m + np.log(np.exp(sc - m[:, None, :]).sum(axis=1))
        alpha = np.where(mask[:, t, None], nxt, alpha)
    fin = alpha + end_transitions.astype(np.float64)[None]
    m = fin.max(axis=1)
    return m + np.log(np.exp(fin - m[:, None]).sum(axis=1))


def run_device(in_maps, trace=False, **kw):
    nc = _build()
    return bass_utils.run_bass_kernel_spmd(
        nc, in_maps, core_ids=list(range(NCORES)), trace=trace, **kw)


def make_in_maps(emissions, transitions, start_transitions, end_transitions):
    ts = _plan()
    tr = np.ascontiguousarray(transitions, dtype=np.float32)
    sv = np.ascontiguousarray(start_transitions, dtype=np.float32).reshape(C, 1)
    ev = np.ascontiguousarray(end_transitions, dtype=np.float32).reshape(C, 1)
    in_maps = []
    for k in range(NCORES):
        sl = slice(k * BLOC, (k + 1) * BLOC)
        em_k = emissions[sl].transpose(2, 1, 0)          # (C, T, BLOC)
        # (C, S, NL+1, BLOC) -> (C, NL+1, S, BLOC) -> (C, NL+1, S*BLOC)
        em_g = em_k[:, ts, :].transpose(0, 2, 1, 3).reshape(C, NL + 1, S * BLOC)
        em_g = np.ascontiguousarray(em_g.astype(ml_dtypes.bfloat16))
        in_maps.append({"em": em_g, "trans": tr, "startv": sv, "endv": ev})
    return in_maps


def kernel(**inputs):
    emissions = np.asarray(inputs["emissions"], dtype=np.float32)
    tags = np.asarray(inputs["tags"]).astype(np.int64)
    mask = np.asarray(inputs["mask"]).astype(bool)
    transitions = np.asarray(inputs["transitions"], dtype=np.float32)
    start_transitions = np.asarray(inputs["start_transitions"], dtype=np.float32)
    end_transitions = np.asarray(inputs["end_transitions"], dtype=np.float32)

    gold = _gold_np(emissions, tags, mask, transitions,
                    start_transitions, end_transitions)

    if mask.all():
        in_maps = make_in_maps(emissions, transitions,
                               start_transitions, end_transitions)
        res = run_device(in_maps)
        logz = np.empty(B, dtype=np.float64)
        for k in range(NCORES):
            caps = np.asarray(res.results[k]["caps"], dtype=np.float64)
            sc = caps[0].reshape(S, BLOC)
            ec = caps[1].reshape(S, BLOC)
            # telescoped segment growths; E carries exp(-MU) on each of the
            # 1023 real steps
            lz = np.log(ec).sum(0) - np.log(sc[1:]).sum(0) + MU * (T - 1)
            logz[k * BLOC:(k + 1) * BLOC] = lz
    else:
        logz = _logz_host(emissions, mask, transitions,
                          start_transitions, end_transitions)

    loss = np.mean(logz - gold)
    return np.asarray(loss, dtype=np.float32)


# revision 17
# speedup vs baseline: 7.2929x; 1.1088x over previous
"""Linear-chain CRF loss (mean over batch of logZ - gold_score) on 8 TRN2 cores.

Math: the forward (alpha) recursion runs in the exp domain:
    a_t = ee_t * (E^T a_{t-1}),   E = exp(transitions - MU),  ee = exp(emissions)
logZ = log(eend^T a_{T-1}) + MU*(T-1).

Key restructure — segmented scan with warmup: products of positive matrices
forget their initial direction at the Birkhoff contraction rate, measured here
at ~0.15x per step (bf16 noise floor ~5e-3 Hilbert reached after ~6 steps).
The serial T-1 = 1023-step chain is split into S=32 segments; each segment's
start direction u_s is recovered by warming up W=6..7 steps from a uniform
vector using the true preceding emissions.  The exact telescoping identity
    logZ = sum_s log(1^T P_s u_s) - sum_{s>=1} log(1^T u_s)
(segment 0 starts from the exact a_0; exp(end) is folded into the last
segment's final emission slot) makes the answer independent of the u_s scale;
direction error enters once per boundary, ~1e-3 in log units, giving ~1e-5
relative error on the loss (tolerance 2e-2).

All 32 chains run concurrently in lockstep bundles of G=16 chains: per
iteration each bundle does ONE [128x128]@[128x256] matmul (PE) and ONE
[128,256] elementwise multiply (DVE), amortizing the fixed PSUM-access cost
over 256 columns.  This converts the latency-bound serial recursion
(~535ns/step) into a throughput-bound pipeline (~24ns/step/chain).

Host precomputes all constants in the exp domain (exp(trans-MU) bf16, the
exact a_0 = exp(em_0 + start), end folded into the last emission) so the
device prologue is just DMA -> exp(emissions) -> go.

Scheduling: raw Bass with manual semaphores (one wait per instruction, so
instructions pre-decode into engine wait queues instead of blocking
sequencers).  The per-step multiply must run on DVE: GPSIMD/Pool cannot
access PSUM, and the Activation engine only supports per-partition scalars.

Sharding: data-parallel over batch, 16 sequences per core, no collectives;
host computes the (tiny) gold path score, capture logs, and the final mean.
"""

import numpy as np
from contextlib import ExitStack

import ml_dtypes
import concourse.bass as bass
import concourse.bacc as bacc
import concourse.mybir as mybir
from concourse import bass_utils

B, T, C = 128, 1024, 128
NCORES = 8
BLOC = B // NCORES            # 16 sequences per core
S = 32                        # time segments (chains) per core
G = 16                        # chains per lockstep bundle
NB = S // G                   # bundles
CW = G * BLOC                 # bundle width (matmul/mul free dim) = 256
NL = 38                       # lockstep iterations per chain
# Segment lengths: chain 0 starts from the exact a_0 (no warmup); the rest
# warm up for NL - L_k steps (6 or 7).  sum(LS) = T-1 = 1023.
LS = [38] + [32] * 24 + [31] * 7
WS = [NL - L for L in LS]     # warmup iterations per chain
# Emission chunk sizes in slots (iteration i consumes slot i); small leading
# chunks let the pipeline start as soon as the first slots are expd.
CS = [1, 2, 3, 4, 5, 5, 5, 5, 5, 3]
NCHUNK = len(CS)
_CHUNK_OF = []                # slot -> chunk index
_CHUNK_BASE = []              # chunk -> first slot
for _ci, _n in enumerate(CS):
    _CHUNK_BASE.append(len(_CHUNK_OF))
    _CHUNK_OF += [_ci] * _n
assert len(_CHUNK_OF) == NL
MU = 5.9                      # per-step log-growth pre-subtraction
RA = 4                        # state-tile ring depth per bundle
NP = 3                        # PSUM-tile ring depth per bundle

F32 = mybir.dt.float32
BF16 = mybir.dt.bfloat16
AF = mybir.ActivationFunctionType

_cache = {}


def _plan():
    """ts[k][i] = original timestep whose emission iteration i applies for
    chain k (warmup uses the true preceding emissions)."""
    bs = np.concatenate([[1], 1 + np.cumsum(LS)])[:S]  # segment starts
    ts = np.zeros((S, NL), dtype=np.int64)
    for k in range(S):
        ts[k, :] = (bs[k] - WS[k]) + np.arange(NL)
    return ts


def _build():
    if "nc" in _cache:
        return _cache["nc"]
    nc = bacc.Bacc("TRN2", target_bir_lowering=False, debug=False)
    em = nc.dram_tensor("em", (C, NL, S * BLOC), BF16, kind="ExternalInput")
    ef = nc.dram_tensor("ef", (C, C), BF16, kind="ExternalInput")
    a0 = nc.dram_tensor("a0", (C, BLOC), BF16, kind="ExternalInput")
    caps = nc.dram_tensor("caps", (2, S * BLOC), F32, kind="ExternalOutput")

    with ExitStack() as ctx:
        _n = iter(range(10 ** 6))
        sbuf = lambda shape, dt: ctx.enter_context(
            nc.sbuf_tensor(f"sb{next(_n)}", shape, dt))
        psum = lambda shape, dt: ctx.enter_context(
            nc.psum_tensor(f"ps{next(_n)}", shape, dt))

        s_dma_c = nc.alloc_semaphore("s_dma_c")   # ef/a0 DMAs (+16 each)
        s_dma_e = nc.alloc_semaphore("s_dma_e")   # emission chunk DMAs
        s_exp = nc.alloc_semaphore("s_exp")       # emission chunk exps
        s_pe = [nc.alloc_semaphore(f"s_pe{g}") for g in range(NB)]
        s_po = [nc.alloc_semaphore(f"s_po{g}") for g in range(NB)]
        s_cap = nc.alloc_semaphore("s_cap")       # capture matmuls
        s_tail = nc.alloc_semaphore("s_tail")
        s_out = nc.alloc_semaphore("s_out")

        Ef = sbuf([C, C], BF16)
        ones_col = sbuf([C, 1], BF16)
        warm = sbuf([1, 1], F32)
        emraw = [sbuf([C, CS[ci], S * BLOC], BF16) for ci in range(NCHUNK)]
        ee = [sbuf([C, CS[ci], S * BLOC], BF16) for ci in range(NCHUNK)]
        st = [[sbuf([C, CW], BF16) for _ in range(RA)] for _ in range(NB)]
        sc_sb = sbuf([1, S * BLOC], F32)
        ec_sb = sbuf([1, S * BLOC], F32)

        ps = [[psum([C, CW], F32) for _ in range(NP)] for _ in range(NB)]
        scap = psum([1, S * BLOC], F32)
        ecap = psum([1, S * BLOC], F32)

        def ee_sl(g, slot):
            ci = _CHUNK_OF[slot]
            return ee[ci][:, slot - _CHUNK_BASE[ci], g * CW:(g + 1) * CW]

        # --- SP queue: input DMAs (first emission chunk leads) ---
        nc.sync.dma_start(
            out=emraw[0][:], in_=em[:, 0:CS[0], :]).then_inc(s_dma_e, 16)
        nc.sync.dma_start(out=Ef[:, :], in_=ef[:, :]).then_inc(s_dma_c, 16)
        nc.sync.dma_start(
            out=st[0][0][:, 0:BLOC], in_=a0[:, :]).then_inc(s_dma_c, 16)
        for ci in range(1, NCHUNK):
            base = _CHUNK_BASE[ci]
            nc.sync.dma_start(
                out=emraw[ci][:], in_=em[:, base:base + CS[ci], :]
            ).then_inc(s_dma_e, 16)

        # --- Act queue: dummy exp first so the Exp table loads during DMA ---
        nc.scalar.activation(warm[:], warm[:], AF.Exp)
        for ci in range(NCHUNK):
            nc.scalar.wait_ge(s_dma_e, 16 * (ci + 1))
            nc.scalar.activation(ee[ci][:], emraw[ci][:], AF.Exp).then_inc(s_exp, 1)

        # --- DVE queue: uniform warmup inits (chain 0's slice arrives by DMA)
        nc.vector.memset(ones_col[:], 1.0)
        nc.vector.memset(st[0][0][:, BLOC:CW], 1.0).then_inc(s_po[0], 1)
        nc.vector.memset(st[1][0][:], 1.0).then_inc(s_po[1], 1)

        # --- PE prologue: Ef and a0 landed ---
        nc.tensor.wait_ge(s_dma_c, 32)

        def runs_of(ks):
            """Group chain ids into (bundle, c_lo, c_hi) runs of adjacent
            columns so each capture is one wide matmul."""
            out = []
            for k in sorted(ks):
                g, c = divmod(k, G)
                if out and out[-1][0] == g and out[-1][2] == c - 1:
                    out[-1][2] = c
                else:
                    out.append([g, c, c])
            return out

        cap_at = {}
        for k in range(S):
            cap_at.setdefault(WS[k], []).append(k)
        ncap = 0

        # --- main loop: NL lockstep iterations over NB bundles ---
        # x_i lives in st[g][i % RA]; TT(i) maps x_i -> x_{i+1}.
        #   mm(g,i)  waits s_po[g] >= i+1, incs s_pe[g]
        #   TT(g,i)  waits s_pe[g] >= i+1, incs s_po[g]
        for i in range(NL):
            if i == 0 or _CHUNK_OF[i] != _CHUNK_OF[i - 1]:
                nc.vector.wait_ge(s_exp, _CHUNK_OF[i] + 1)
            for g in range(NB):
                nc.tensor.matmul(
                    ps[g][i % NP][:], Ef[:], st[g][i % RA][:], start=True, stop=True
                )._wait_ge(s_po[g], i + 1).then_inc(s_pe[g], 1)
            # start-captures: 1^T u_k for chains whose warmup ends here
            for g, c0, c1 in runs_of(cap_at.get(i, [])):
                nc.tensor.matmul(
                    scap[:, (g * G + c0) * BLOC:(g * G + c1 + 1) * BLOC],
                    ones_col[:],
                    st[g][i % RA][:, c0 * BLOC:(c1 + 1) * BLOC],
                    start=True, stop=True,
                )._wait_ge(s_po[g], i + 1).then_inc(s_cap, 1)
                ncap += 1
            for g in range(NB):
                nc.vector.tensor_mul(
                    st[g][(i + 1) % RA][:], ps[g][i % NP][:], ee_sl(g, i)
                )._wait_ge(s_pe[g], i + 1).then_inc(s_po[g], 1)

        nscap = ncap  # start-captures emitted so far

        # --- end-captures: 1^T x_NL per chain (end folded into emissions) ---
        for g in range(NB):
            nc.tensor.matmul(
                ecap[:, g * CW:(g + 1) * CW], ones_col[:],
                st[g][NL % RA][:], start=True, stop=True,
            )._wait_ge(s_po[g], NL + 1).then_inc(s_cap, 1)
            ncap += 1

        # --- tail: copy captures PSUM->SBUF on Act (DMA can't read PSUM);
        # start-captures copied as soon as they're all done (mid-loop) ---
        nc.scalar.activation(
            sc_sb[:], scap[:], AF.Copy)._wait_ge(s_cap, nscap).then_inc(s_tail, 1)
        nc.scalar.activation(
            ec_sb[:], ecap[:], AF.Copy)._wait_ge(s_cap, ncap).then_inc(s_tail, 1)
        nc.sync.wait_ge(s_tail, 1)
        nc.sync.dma_start(out=caps[0:1, :], in_=sc_sb[:]).then_inc(s_out, 16)
        nc.sync.wait_ge(s_tail, 2)
        nc.sync.dma_start(out=caps[1:2, :], in_=ec_sb[:]).then_inc(s_out, 16)

    nc.compile()
    _cache["nc"] = nc
    return nc


def _gold_np(emissions, tags, mask, transitions, start_transitions, end_transitions):
    em = emissions.astype(np.float64)
    mf = mask.astype(np.float64)
    idx = np.arange(B)
    emit = np.take_along_axis(em, tags[:, :, None], axis=2)[:, :, 0]
    tr = transitions.astype(np.float64)[tags[:, :-1], tags[:, 1:]]
    score = start_transitions.astype(np.float64)[tags[:, 0]] + emit[:, 0]
    score = score + np.sum((emit[:, 1:] + tr) * mf[:, 1:], axis=1)
    last_idx = mask.astype(np.int64).sum(axis=1) - 1
    last_tags = tags[idx, last_idx]
    return score + end_transitions.astype(np.float64)[last_tags]


def _logz_host(emissions, mask, transitions, start_transitions, end_transitions):
    # Slow exact fallback (only for non-all-ones masks, which the spec never
    # produces).
    em = emissions.astype(np.float64)
    tr = transitions.astype(np.float64)
    alpha = start_transitions.astype(np.float64) + em[:, 0]
    for t in range(1, T):
        sc = alpha[:, :, None] + tr[None] + em[:, t, None, :]
        m = sc.max(axis=1)
        nxt = m + np.log(np.exp(sc - m[:, None, :]).sum(axis=1))
        alpha = np.where(mask[:, t, None], nxt, alpha)
    fin = alpha + end_transitions.astype(np.float64)[None]
    m = fin.max(axis=1)
    return m + np.log(np.exp(fin - m[:, None]).sum(axis=1))


def run_device(in_maps, trace=False, **kw):
    nc = _build()
    return bass_utils.run_bass_kernel_spmd(
        nc, in_maps, core_ids=list(range(NCORES)), trace=trace, **kw)


def make_in_maps(emissions, transitions, start_transitions, end_transitions):
    ts = _plan()
    ef = np.exp(transitions.astype(np.float64) - MU).astype(ml_dtypes.bfloat16)
    ef = np.ascontiguousarray(ef)
    in_maps = []
    for k in range(NCORES):
        sl = slice(k * BLOC, (k + 1) * BLOC)
        em_k = emissions[sl].transpose(2, 1, 0).astype(np.float64)  # (C,T,BLOC)
        # exact a_0 = exp(em_0 + start)
        a0 = np.exp(em_k[:, 0, :] + start_transitions.astype(np.float64)[:, None])
        # (C, S, NL, BLOC): per-chain emission streams (warmup + segment)
        em_g = em_k[:, ts, :]
        # fold exp(end) into the last chain's final slot
        em_g[:, S - 1, NL - 1, :] += end_transitions.astype(np.float64)[:, None]
        em_g = em_g.transpose(0, 2, 1, 3).reshape(C, NL, S * BLOC)
        in_maps.append({
            "em": np.ascontiguousarray(em_g.astype(ml_dtypes.bfloat16)),
            "ef": ef,
            "a0": np.ascontiguousarray(a0.astype(ml_dtypes.bfloat16)),
        })
    return in_maps


def kernel(**inputs):
    emissions = np.asarray(inputs["emissions"], dtype=np.float32)
    tags = np.asarray(inputs["tags"]).astype(np.int64)
    mask = np.asarray(inputs["mask"]).astype(bool)
    transitions = np.asarray(inputs["transitions"], dtype=np.float32)
    start_transitions = np.asarray(inputs["start_transitions"], dtype=np.float32)
    end_transitions = np.asarray(inputs["end_transitions"], dtype=np.float32)

    gold = _gold_np(emissions, tags, mask, transitions,
                    start_transitions, end_transitions)

    if mask.all():
        in_maps = make_in_maps(emissions, transitions,
                               start_transitions, end_transitions)
        res = run_device(in_maps)
        logz = np.empty(B, dtype=np.float64)
        for k in range(NCORES):
            caps = np.asarray(res.results[k]["caps"], dtype=np.float64)
            sc = caps[0].reshape(S, BLOC)
            ec = caps[1].reshape(S, BLOC)
            # telescoped segment growths; E carries exp(-MU) on each of the
            # 1023 real steps
            lz = np.log(ec).sum(0) - np.log(sc[1:]).sum(0) + MU * (T - 1)
            logz[k * BLOC:(k + 1) * BLOC] = lz
    else:
        logz = _logz_host(emissions, mask, transitions,
                          start_transitions, end_transitions)

    loss = np.mean(logz - gold)
    return np.asarray(loss, dtype=np.float32)


# revision 20
# speedup vs baseline: 7.5979x; 1.0418x over previous
"""Linear-chain CRF loss (mean over batch of logZ - gold_score) on 8 TRN2 cores.

Math: the forward (alpha) recursion runs in the exp domain:
    a_t = ee_t * (E^T a_{t-1}),   E = exp(transitions - MU),  ee = exp(emissions)
logZ = log(eend^T a_{T-1}) + MU*(T-1).

Key restructure — segmented scan with warmup: products of positive matrices
forget their initial direction at the Birkhoff contraction rate, measured here
at ~0.15x per step (bf16 noise floor ~5e-3 Hilbert reached after ~6 steps).
The serial T-1 = 1023-step chain is split into S=32 segments; each segment's
start direction u_s is recovered by warming up W=6..7 steps from a uniform
vector using the true preceding emissions.  The exact telescoping identity
    logZ = sum_s log(1^T P_s u_s) - sum_{s>=1} log(1^T u_s)
(segment 0 starts from the exact a_0; exp(end) is folded into the last
segment's final emission slot) makes the answer independent of the u_s scale;
direction error enters once per boundary, ~1e-3 in log units, giving ~1e-5
relative error on the loss (tolerance 2e-2).

All 32 chains run concurrently in lockstep bundles of G=16 chains: per
iteration each bundle does ONE [128x128]@[128x256] matmul (PE) and ONE
[128,256] elementwise multiply (DVE), amortizing the fixed PSUM-access cost
over 256 columns.  This converts the latency-bound serial recursion
(~535ns/step) into a throughput-bound pipeline (~24ns/step/chain).

Host precomputes all constants in the exp domain (exp(trans-MU) bf16, the
exact a_0 = exp(em_0 + start), end folded into the last emission) so the
device prologue is just DMA -> exp(emissions) -> go.

Scheduling: raw Bass with manual semaphores (one wait per instruction, so
instructions pre-decode into engine wait queues instead of blocking
sequencers).  The per-step multiply must run on DVE: GPSIMD/Pool cannot
access PSUM, and the Activation engine only supports per-partition scalars.

Sharding: data-parallel over batch, 16 sequences per core, no collectives;
host computes the (tiny) gold path score, capture logs, and the final mean.
"""

import numpy as np
from contextlib import ExitStack

import ml_dtypes
import concourse.bass as bass
import concourse.bacc as bacc
import concourse.mybir as mybir
from concourse import bass_utils

B, T, C = 128, 1024, 128
NCORES = 8
BLOC = B // NCORES            # 16 sequences per core
S = 32                        # time segments (chains) per core
G = 16                        # chains per lockstep bundle
NB = S // G                   # bundles
CW = G * BLOC                 # bundle width (matmul/mul free dim) = 256
NL = 38                       # lockstep iterations per chain
# Segment lengths: chain 0 starts from the exact a_0 (no warmup); the rest
# warm up for NL - L_k steps (6 or 7).  sum(LS) = T-1 = 1023.
LS = [38] + [32] * 24 + [31] * 7
WS = [NL - L for L in LS]     # warmup iterations per chain
# Emission chunk sizes in slots (iteration i consumes slot i); small leading
# chunks let the pipeline start as soon as the first slots are expd.
CS = [1, 2, 3, 4, 5, 5, 5, 5, 5, 3]
NCHUNK = len(CS)
_CHUNK_OF = []                # slot -> chunk index
_CHUNK_BASE = []              # chunk -> first slot
for _ci, _n in enumerate(CS):
    _CHUNK_BASE.append(len(_CHUNK_OF))
    _CHUNK_OF += [_ci] * _n
assert len(_CHUNK_OF) == NL
MU = 5.9                      # per-step log-growth pre-subtraction
RA = 4                        # state-tile ring depth per bundle
NP = 3                        # PSUM-tile ring depth per bundle

F32 = mybir.dt.float32
BF16 = mybir.dt.bfloat16
AF = mybir.ActivationFunctionType

_cache = {}


def _plan():
    """ts[k][i] = original timestep whose emission iteration i applies for
    chain k (warmup uses the true preceding emissions)."""
    bs = np.concatenate([[1], 1 + np.cumsum(LS)])[:S]  # segment starts
    ts = np.zeros((S, NL), dtype=np.int64)
    for k in range(S):
        ts[k, :] = (bs[k] - WS[k]) + np.arange(NL)
    return ts


def _build():
    if "nc" in _cache:
        return _cache["nc"]
    nc = bacc.Bacc("TRN2", target_bir_lowering=False, debug=False)
    em = nc.dram_tensor("em", (C, NL, S * BLOC), BF16, kind="ExternalInput")
    ef = nc.dram_tensor("ef", (C, C), BF16, kind="ExternalInput")
    a0 = nc.dram_tensor("a0", (C, BLOC), BF16, kind="ExternalInput")
    caps = nc.dram_tensor("caps", (2, S * BLOC), F32, kind="ExternalOutput")

    with ExitStack() as ctx:
        _n = iter(range(10 ** 6))
        sbuf = lambda shape, dt: ctx.enter_context(
            nc.sbuf_tensor(f"sb{next(_n)}", shape, dt))
        psum = lambda shape, dt: ctx.enter_context(
            nc.psum_tensor(f"ps{next(_n)}", shape, dt))

        s_dma_c = nc.alloc_semaphore("s_dma_c")   # ef/a0 DMAs (+16 each)
        s_dma_e = nc.alloc_semaphore("s_dma_e")   # emission chunk DMAs
        s_exp = nc.alloc_semaphore("s_exp")       # emission chunk exps
        s_pe = [nc.alloc_semaphore(f"s_pe{g}") for g in range(NB)]
        s_po = [nc.alloc_semaphore(f"s_po{g}") for g in range(NB)]
        s_cap = nc.alloc_semaphore("s_cap")       # capture matmuls
        s_tail = nc.alloc_semaphore("s_tail")
        s_out = nc.alloc_semaphore("s_out")

        Ef = sbuf([C, C], BF16)
        ones_col = sbuf([C, 1], BF16)
        warm = sbuf([1, 1], F32)
        emraw = [sbuf([C, CS[ci], S * BLOC], BF16) for ci in range(NCHUNK)]
        ee = [sbuf([C, CS[ci], S * BLOC], BF16) for ci in range(NCHUNK)]
        st = [[sbuf([C, CW], BF16) for _ in range(RA)] for _ in range(NB)]
        sc_sb = sbuf([1, S * BLOC], F32)
        ec_sb = sbuf([1, S * BLOC], F32)

        ps = [[psum([C, CW], F32) for _ in range(NP)] for _ in range(NB)]
        scap = psum([1, S * BLOC], F32)
        ecap = psum([1, S * BLOC], F32)

        def ee_sl(g, slot):
            ci = _CHUNK_OF[slot]
            return ee[ci][:, slot - _CHUNK_BASE[ci], g * CW:(g + 1) * CW]

        # --- SP queue: emission chunk DMAs only (consts go via Act's DGE) ---
        for ci in range(NCHUNK):
            base = _CHUNK_BASE[ci]
            nc.sync.dma_start(
                out=emraw[ci][:], in_=em[:, base:base + CS[ci], :]
            ).then_inc(s_dma_e, 16)

        # --- Act queue: dummy exp first so the Exp table loads during DMA;
        # Ef/a0 ride Act's separate HWDGE unit in parallel with chunk 0 ---
        nc.scalar.activation(warm[:], warm[:], AF.Exp)
        nc.scalar.dma_start(out=Ef[:, :], in_=ef[:, :]).then_inc(s_dma_c, 16)
        nc.scalar.dma_start(
            out=st[0][0][:, 0:BLOC], in_=a0[:, :]).then_inc(s_dma_c, 16)
        for ci in range(NCHUNK):
            nc.scalar.wait_ge(s_dma_e, 16 * (ci + 1))
            nc.scalar.activation(ee[ci][:], emraw[ci][:], AF.Exp).then_inc(s_exp, 1)

        # --- DVE queue: uniform warmup inits (chain 0's slice arrives by DMA)
        nc.vector.memset(ones_col[:], 1.0)
        nc.vector.memset(st[0][0][:, BLOC:CW], 1.0).then_inc(s_po[0], 1)
        nc.vector.memset(st[1][0][:], 1.0).then_inc(s_po[1], 1)

        # --- PE prologue: p-state warmers (results unread), then input gate
        for w in range(12):
            nc.tensor.matmul(
                ps[0][w % NP][:], st[1][1][:, 0:C], st[1][1][:],
                start=True, stop=True)
        nc.tensor.wait_ge(s_dma_c, 32)

        def runs_of(ks):
            """Group chain ids into (bundle, c_lo, c_hi) runs of adjacent
            columns so each capture is one wide matmul."""
            out = []
            for k in sorted(ks):
                g, c = divmod(k, G)
                if out and out[-1][0] == g and out[-1][2] == c - 1:
                    out[-1][2] = c
                else:
                    out.append([g, c, c])
            return out

        cap_at = {}
        for k in range(S):
            cap_at.setdefault(WS[k], []).append(k)
        ncap = 0

        # --- main loop: NL lockstep iterations over NB bundles ---
        # x_i lives in st[g][i % RA]; TT(i) maps x_i -> x_{i+1}.
        #   mm(g,i)  waits s_po[g] >= i+1, incs s_pe[g]
        #   TT(g,i)  waits s_pe[g] >= i+1, incs s_po[g]
        for i in range(NL):
            if i == 0 or _CHUNK_OF[i] != _CHUNK_OF[i - 1]:
                nc.vector.wait_ge(s_exp, _CHUNK_OF[i] + 1)
            for g in range(NB):
                nc.tensor.matmul(
                    ps[g][i % NP][:], Ef[:], st[g][i % RA][:], start=True, stop=True
                )._wait_ge(s_po[g], i + 1).then_inc(s_pe[g], 1)
            # start-captures: 1^T u_k for chains whose warmup ends here
            for g, c0, c1 in runs_of(cap_at.get(i, [])):
                nc.tensor.matmul(
                    scap[:, (g * G + c0) * BLOC:(g * G + c1 + 1) * BLOC],
                    ones_col[:],
                    st[g][i % RA][:, c0 * BLOC:(c1 + 1) * BLOC],
                    start=True, stop=True,
                )._wait_ge(s_po[g], i + 1).then_inc(s_cap, 1)
                ncap += 1
            for g in range(NB):
                nc.vector.tensor_mul(
                    st[g][(i + 1) % RA][:], ps[g][i % NP][:], ee_sl(g, i)
                )._wait_ge(s_pe[g], i + 1).then_inc(s_po[g], 1)

        nscap = ncap  # start-captures emitted so far

        # --- end-captures: 1^T x_NL per chain (end folded into emissions) ---
        for g in range(NB):
            nc.tensor.matmul(
                ecap[:, g * CW:(g + 1) * CW], ones_col[:],
                st[g][NL % RA][:], start=True, stop=True,
            )._wait_ge(s_po[g], NL + 1).then_inc(s_cap, 1)
            ncap += 1

        # --- tail: copy captures PSUM->SBUF on Act (DMA can't read PSUM);
        # start-captures copied as soon as they're all done (mid-loop) ---
        nc.scalar.activation(
            sc_sb[:], scap[:], AF.Copy)._wait_ge(s_cap, nscap).then_inc(s_tail, 1)
        nc.scalar.activation(
            ec_sb[:], ecap[:], AF.Copy)._wait_ge(s_cap, ncap).then_inc(s_tail, 1)
        nc.sync.wait_ge(s_tail, 1)
        nc.sync.dma_start(out=caps[0:1, :], in_=sc_sb[:])
        nc.sync.wait_ge(s_tail, 2)
        nc.sync.dma_start(out=caps[1:2, :], in_=ec_sb[:])

    nc.compile()
    _cache["nc"] = nc
    return nc


def _gold_np(emissions, tags, mask, transitions, start_transitions, end_transitions):
    em = emissions.astype(np.float64)
    mf = mask.astype(np.float64)
    idx = np.arange(B)
    emit = np.take_along_axis(em, tags[:, :, None], axis=2)[:, :, 0]
    tr = transitions.astype(np.float64)[tags[:, :-1], tags[:, 1:]]
    score = start_transitions.astype(np.float64)[tags[:, 0]] + emit[:, 0]
    score = score + np.sum((emit[:, 1:] + tr) * mf[:, 1:], axis=1)
    last_idx = mask.astype(np.int64).sum(axis=1) - 1
    last_tags = tags[idx, last_idx]
    return score + end_transitions.astype(np.float64)[last_tags]


def _logz_host(emissions, mask, transitions, start_transitions, end_transitions):
    # Slow exact fallback (only for non-all-ones masks, which the spec never
    # produces).
    em = emissions.astype(np.float64)
    tr = transitions.astype(np.float64)
    alpha = start_transitions.astype(np.float64) + em[:, 0]
    for t in range(1, T):
        sc = alpha[:, :, None] + tr[None] + em[:, t, None, :]
        m = sc.max(axis=1)
        nxt = m + np.log(np.exp(sc - m[:, None, :]).sum(axis=1))
        alpha = np.where(mask[:, t, None], nxt, alpha)
    fin = alpha + end_transitions.astype(np.float64)[None]
    m = fin.max(axis=1)
    return m + np.log(np.exp(fin - m[:, None]).sum(axis=1))


def run_device(in_maps, trace=False, **kw):
    nc = _build()
    return bass_utils.run_bass_kernel_spmd(
        nc, in_maps, core_ids=list(range(NCORES)), trace=trace, **kw)


def make_in_maps(emissions, transitions, start_transitions, end_transitions):
    ts = _plan()
    ef = np.exp(transitions.astype(np.float64) - MU).astype(ml_dtypes.bfloat16)
    ef = np.ascontiguousarray(ef)
    in_maps = []
    for k in range(NCORES):
        sl = slice(k * BLOC, (k + 1) * BLOC)
        em_k = emissions[sl].transpose(2, 1, 0).astype(np.float64)  # (C,T,BLOC)
        # exact a_0 = exp(em_0 + start)
        a0 = np.exp(em_k[:, 0, :] + start_transitions.astype(np.float64)[:, None])
        # (C, S, NL, BLOC): per-chain emission streams (warmup + segment)
        em_g = em_k[:, ts, :]
        # fold exp(end) into the last chain's final slot
        em_g[:, S - 1, NL - 1, :] += end_transitions.astype(np.float64)[:, None]
        em_g = em_g.transpose(0, 2, 1, 3).reshape(C, NL, S * BLOC)
        in_maps.append({
            "em": np.ascontiguousarray(em_g.astype(ml_dtypes.bfloat16)),
            "ef": ef,
            "a0": np.ascontiguousarray(a0.astype(ml_dtypes.bfloat16)),
        })
    return in_maps


def kernel(**inputs):
    emissions = np.asarray(inputs["emissions"], dtype=np.float32)
    tags = np.asarray(inputs["tags"]).astype(np.int64)
    mask = np.asarray(inputs["mask"]).astype(bool)
    transitions = np.asarray(inputs["transitions"], dtype=np.float32)
    start_transitions = np.asarray(inputs["start_transitions"], dtype=np.float32)
    end_transitions = np.asarray(inputs["end_transitions"], dtype=np.float32)

    gold = _gold_np(emissions, tags, mask, transitions,
                    start_transitions, end_transitions)

    if mask.all():
        in_maps = make_in_maps(emissions, transitions,
                               start_transitions, end_transitions)
        res = run_device(in_maps)
        logz = np.empty(B, dtype=np.float64)
        for k in range(NCORES):
            caps = np.asarray(res.results[k]["caps"], dtype=np.float64)
            sc = caps[0].reshape(S, BLOC)
            ec = caps[1].reshape(S, BLOC)
            # telescoped segment growths; E carries exp(-MU) on each of the
            # 1023 real steps
            lz = np.log(ec).sum(0) - np.log(sc[1:]).sum(0) + MU * (T - 1)
            logz[k * BLOC:(k + 1) * BLOC] = lz
    else:
        logz = _logz_host(emissions, mask, transitions,
                          start_transitions, end_transitions)

    loss = np.mean(logz - gold)
    return np.asarray(loss, dtype=np.float32)
